# revision 1
# baseline (speedup 1.0000x reference)
"""DaVinci attention (multi-modal MoE-routed attention block) on 8 Trainium2
NeuronCores.

Sharding: tensor-parallel over heads.  Each of the 8 cores owns one KV head
and its 5 GQA query heads: qkv-weight columns (640 q + 128 k + 128 v + 5 gate
per core) and proj-weight rows (640 per core) are sliced per core; the final
projection output is a partial sum reduced on the host.

Host-side prep (layout only — all FLOPs stay on device):
  * tokens are permuted so same-modality tokens are contiguous; each expert's
    GEMM then runs on its own token range (no 3x masked-dispatch waste)
  * pre-norm weight (w+1) is folded into the qkv weight columns; the
    per-token rms scale is applied on-device after the GEMM
  * q/k-norm weights (w+1) are folded into host-precomputed rope coefficient
    tables A=cos*(w1+1), B=sin*(w2+1), D=sin*(w1+1), E=cos*(w2+1)
  * weights are pre-transposed/tiled for contraction-major DMA
"""

import os
import sys
import types

import numpy as np

HIDDEN = 5120
HEAD_DIM = 128
HQ = 40
HKV = 8
NUM_MOD = 3
Q_SIZE = HQ * HEAD_DIM          # 5120
KV_SIZE = HKV * HEAD_DIM        # 1024
GATE = HQ
QKV_OUT = Q_SIZE + 2 * KV_SIZE + GATE  # 7208
EPS = 1e-6
N_TOK = 2048
P = 128
NCORES = 8
GQ = HQ // HKV                  # 5 q heads per core
QC = GQ * HEAD_DIM              # 640 q cols per core
FC = QC + 2 * HEAD_DIM + GQ     # 901 qkv out features per core
KO = HIDDEN // P                # 40 contraction chunks
NB = N_TOK // P                 # 16 token blocks of 128 (attention tiling)
N2 = 1024                       # attention free-dim chunk
SCALE = 1.0 / float(np.sqrt(HEAD_DIM))

LAST_EXEC_NS = None             # filled when BASSMOE_TRACE=1


# ---------------------------------------------------------------------------
# axon NTFF profiling hook (needed only when tracing) + BIR sync legalizer
# ---------------------------------------------------------------------------

def _install_profile_hook():
    if "antenv.axon_hooks" in sys.modules:
        return
    mod = types.ModuleType("antenv.axon_hooks")
    _h = [None]
    mod.set_axon_ntff_profile_hook = lambda h: _h.__setitem__(0, h)
    mod.get_axon_ntff_profile_hook = lambda: _h[0]
    import antenv

    antenv.axon_hooks = mod
    sys.modules["antenv.axon_hooks"] = mod
    try:
        from trn_agent_boot.trn_boot import _ntff_profile_via_ctypes

        mod.set_axon_ntff_profile_hook(
            _ntff_profile_via_ctypes("/opt/axon/libaxon_pjrt.so")
        )
    except Exception:
        pass


def _legalize_sync(bir_json):
    """This walrus build accepts a single sync wait/update per instruction.
    Move extra waits onto preceding same-engine NoOps (the engine stalls
    before dispatch either way) and extra updates onto trailing NoOps."""
    import json

    data = json.loads(bir_json)
    for fn in data["functions"]:
        for blk in fn["blocks"]:
            out = []
            for ins in blk["instructions"]:
                si = ins.get("sync_info")
                waits = si.get("on_wait", []) if si else []
                upds = si.get("on_update", []) if si else []
                if len(waits) > 1:
                    for i, w in enumerate(waits[:-1]):
                        out.append({
                            "debug": ins.get("debug", 0),
                            "engine": ins["engine"],
                            "ins": [], "is_reset_sema": False,
                            "name": f"{ins['name']}-lw{i}",
                            "opcode": "NoOp", "outs": [],
                            "sync_info": {"on_update": [], "on_wait": [w]},
                        })
                    si["on_wait"] = [waits[-1]]
                out.append(ins)
                if len(upds) > 1:
                    if ins["opcode"] in ("DMACopy", "DMATranspose"):
                        raise AssertionError(
                            f"DMA instruction {ins['name']} has multiple updates")
                    for i, u in enumerate(upds[1:]):
                        out.append({
                            "debug": ins.get("debug", 0),
                            "engine": ins["engine"],
                            "ins": [], "is_reset_sema": False,
                            "name": f"{ins['name']}-lu{i}",
                            "opcode": "NoOp", "outs": [],
                            "sync_info": {"on_update": [u], "on_wait": []},
                        })
                    si["on_update"] = [upds[0]]
            blk["instructions"] = out
    return json.dumps(data).encode()


def _install_legalizer():
    from concourse import bass2jax, bass_utils

    if getattr(bass2jax, "_sync_legalize_installed", False):
        return
    orig = bass_utils.compile_bir_kernel

    def wrapped(bir_json, tmpdir, neff_name="file.neff"):
        return orig(_legalize_sync(bir_json), tmpdir, neff_name)

    bass2jax.compile_bir_kernel = wrapped
    bass_utils.compile_bir_kernel = wrapped
    bass2jax._sync_legalize_installed = True


# ---------------------------------------------------------------------------
# device program
# ---------------------------------------------------------------------------

_BUILD_CACHE = {}


def _subranges(lo, hi, starts):
    """Split [lo,hi) by the group boundaries in `starts` (len 4, cumulative).
    Yields (a, b, g) with lo<=a<b<=hi."""
    out = []
    for g in range(3):
        a = max(lo, starts[g])
        b = min(hi, starts[g + 1])
        if a < b:
            out.append((a, b, g))
    return out


def _build(counts):
    import concourse.bass as bass
    import concourse.tile as tile
    from concourse import mybir
    from concourse.masks import make_identity

    f32 = mybir.dt.float32
    bf16 = mybir.dt.bfloat16
    AF = mybir.ActivationFunctionType
    OP = mybir.AluOpType

    n0, n1, n2 = counts
    starts = [0, n0, n0 + n1, 2048]
    # group-chunked qkv tiles (tok0, nt, g)
    tiles = []
    for g in range(3):
        t0, t1 = starts[g], starts[g + 1]
        for a in range(t0, t1, P):
            tiles.append((a, min(P, t1 - a), g))
    # group-chunked proj token chunks (c0, cn, g)
    chunks = []
    for g in range(3):
        t0, t1 = starts[g], starts[g + 1]
        for a in range(t0, t1, 512):
            chunks.append((a, min(512, t1 - a), g))
    # packed-xt flat offsets per tile
    xt_offs = []
    off = 0
    for (a, nt, g) in tiles:
        xt_offs.append(off)
        off += P * KO * nt
    xt_total = off

    nc = bass.Bass()
    xt = nc.dram_tensor("xt", (xt_total,), bf16, kind="ExternalInput")
    xn = nc.dram_tensor("xn", (N_TOK, HIDDEN), bf16, kind="ExternalInput")
    ropec = nc.dram_tensor("ropec", (N_TOK, 8, 64), f32, kind="ExternalInput")
    wqkv = nc.dram_tensor("wqkv", (NUM_MOD, KO, P, FC), bf16, kind="ExternalInput")
    wproj = nc.dram_tensor("wproj", (NUM_MOD, GQ, P, HIDDEN), bf16,
                           kind="ExternalInput")
    outT = nc.dram_tensor("outT", (HIDDEN, N_TOK), f32, kind="ExternalOutput")

    with tile.TileContext(nc) as tc:
        with tc.tile_pool(name="cst", bufs=1) as cst, \
             tc.tile_pool(name="gdram", bufs=1, space="DRAM") as gdram, \
             tc.tile_pool(name="glob", bufs=1) as glob:
            ident = cst.tile([P, P], f32)
            make_identity(nc, ident)
            ident_bf = cst.tile([P, P], bf16)
            make_identity(nc, ident_bf)
            ones_bf = cst.tile([P, 1], bf16)
            nc.vector.memset(ones_bf, 1.0)
            eps_t = cst.tile([P, 1], f32)
            nc.vector.memset(eps_t, EPS)

            # persistent activations
            qkT = glob.tile([P, 6, N_TOK], bf16)     # [d, head(0-4=q,5=k), n]
            vT_g = glob.tile([P, N_TOK], bf16)       # [d, n]
            v_all = glob.tile([P, NB, P], bf16)      # [n%128, n//128, d]
            oT_all = glob.tile([P, GQ, N_TOK], bf16)  # [d, head, n]
            g_sig = glob.tile([8, N_TOK], f32)       # sigmoid(gate) rows

            # ---------------- phase A: rms + qkv GEMM + norms + rope ------
            with tc.tile_pool(name="paw", bufs=1) as paw, \
                 tc.tile_pool(name="pa2", bufs=2) as pa2, \
                 tc.tile_pool(name="pa1", bufs=1) as pa1, \
                 tc.tile_pool(name="psA", bufs=6, space="PSUM") as psA, \
                 tc.tile_pool(name="psT", bufs=2, space="PSUM") as psT:
                KQ = KO // 4            # 10 ko per weight quarter
                for g in range(3):
                    # quarter the group weight so the next group's quarters
                    # stream in under this group's matmuls
                    wq_sb = []
                    for q in range(4):
                        wt = paw.tile([P, KQ, FC], bf16, tag=f"wq{q}")
                        nc.sync.dma_start(
                            out=wt[:],
                            in_=wqkv[g, q * KQ:(q + 1) * KQ]
                            .rearrange("ko p f -> p ko f"))
                        wq_sb.append(wt)
                    for ti, (tok0, nt, gg) in enumerate(tiles):
                        if gg != g:
                            continue
                        xt_t = pa2.tile([P, KO, P], bf16, tag="xt")
                        nc.sync.dma_start(
                            out=xt_t[:, :, :nt],
                            in_=xt[xt_offs[ti]:xt_offs[ti] + P * KO * nt]
                            .rearrange("(p ko j) -> p ko j", p=P, ko=KO))
                        xn_t = pa1.tile([P, HIDDEN], bf16, tag="xn")
                        nc.sync.dma_start(out=xn_t[:nt],
                                          in_=xn[tok0:tok0 + nt])
                        rp_t = pa2.tile([P, 8, 64], f32, tag="rp")
                        nc.sync.dma_start(out=rp_t[:nt],
                                          in_=ropec[tok0:tok0 + nt])
                        # pre-norm rms (from raw x): sum(x^2) via ScalarE
                        # Square+accum, then sqrt(acc/H + eps), reciprocal
                        ssq = pa2.tile([P, 1], f32, tag="ssq")
                        nc.scalar.activation(out=xn_t[:nt], in_=xn_t[:nt],
                                             func=AF.Square,
                                             accum_out=ssq[:nt])
                        srt = pa2.tile([P, 1], f32, tag="srt")
                        nc.scalar.activation(srt[:nt], ssq[:nt], AF.Sqrt,
                                             scale=1.0 / HIDDEN,
                                             bias=eps_t[:nt])
                        rinv = pa2.tile([P, 1], f32, tag="rinv")
                        nc.vector.reciprocal(rinv[:nt], srt[:nt])
                        # qkv GEMM: psum [tokens, features]
                        ps_a = psA.tile([P, 512], f32, tag="ps512")
                        ps_b = psA.tile([P, 512], f32, tag="ps512")
                        for ko in range(KO):
                            wt = wq_sb[ko // KQ]
                            kq = ko % KQ
                            nc.tensor.matmul(
                                ps_a[:nt, :],
                                lhsT=xt_t[:, ko, :nt],
                                rhs=wt[:, kq, 0:512],
                                start=(ko == 0), stop=(ko == KO - 1))
                            nc.tensor.matmul(
                                ps_b[:nt, 0:FC - 512],
                                lhsT=xt_t[:, ko, :nt],
                                rhs=wt[:, kq, 512:FC],
                                start=(ko == 0), stop=(ko == KO - 1))
                        # evacuate with rms scale
                        qf = pa1.tile([P, GQ, HEAD_DIM], f32, tag="qf")
                        kf = pa1.tile([P, HEAD_DIM], f32, tag="kf")
                        vf = pa1.tile([P, HEAD_DIM], bf16, tag="vf")
                        gf = pa1.tile([P, 8], f32, tag="gf")
                        nc.vector.tensor_scalar_mul(
                            qf[:nt, 0:4, :], ps_a[:nt, :], rinv[:nt])
                        nc.vector.tensor_scalar_mul(
                            qf[:nt, 4, :], ps_b[:nt, 0:128], rinv[:nt])
                        nc.vector.tensor_scalar_mul(
                            kf[:nt, :], ps_b[:nt, 128:256], rinv[:nt])
                        nc.vector.tensor_scalar_mul(
                            vf[:nt, :], ps_b[:nt, 256:384], rinv[:nt])
                        nc.vector.tensor_scalar_mul(
                            gf[:nt, 0:GQ], ps_b[:nt, 384:389], rinv[:nt])
                        # q/k rms over head_dim (Square+accum per head)
                        sq = pa2.tile([P, 8], f32, tag="sq")
                        junk = pa1.tile([P, HEAD_DIM], f32, tag="junk")
                        for h in range(GQ):
                            nc.scalar.activation(
                                out=junk[:nt], in_=qf[:nt, h, :],
                                func=AF.Square,
                                accum_out=sq[:nt, h:h + 1])
                        nc.scalar.activation(
                            out=junk[:nt], in_=kf[:nt], func=AF.Square,
                            accum_out=sq[:nt, GQ:GQ + 1])
                        sqs = pa2.tile([P, 8], f32, tag="sqs")
                        nc.scalar.activation(sqs[:nt, 0:6], sq[:nt, 0:6],
                                             AF.Sqrt, scale=1.0 / HEAD_DIM,
                                             bias=eps_t[:nt])
                        rq = pa2.tile([P, 8], f32, tag="rq")
                        nc.vector.reciprocal(rq[:nt, 0:6], sqs[:nt, 0:6])
                        # rope+norm for q (coeff tables already fold w+1)
                        q1 = qf[:nt, :, 0:64]
                        q2 = qf[:nt, :, 64:128]
                        t1 = pa1.tile([P, GQ, 64], f32, tag="t1")
                        t2 = pa1.tile([P, GQ, 64], f32, tag="t2")
                        qr = pa2.tile([P, GQ, HEAD_DIM], f32, tag="qr")

                        def bc(i):
                            return rp_t[:nt, i:i + 1, :].to_broadcast(
                                (nt, GQ, 64))

                        nc.vector.tensor_tensor(t1[:nt], q1, bc(0), OP.mult)
                        nc.vector.tensor_tensor(t2[:nt], q2, bc(1), OP.mult)
                        nc.vector.tensor_tensor(qr[:nt, :, 0:64], t1[:nt],
                                                t2[:nt], OP.subtract)
                        nc.vector.tensor_tensor(t1[:nt], q1, bc(2), OP.mult)
                        nc.vector.tensor_tensor(t2[:nt], q2, bc(3), OP.mult)
                        nc.vector.tensor_tensor(qr[:nt, :, 64:128], t1[:nt],
                                                t2[:nt], OP.add)
                        nc.vector.tensor_tensor(
                            qr[:nt], qr[:nt],
                            rq[:nt, 0:GQ, None].to_broadcast(
                                (nt, GQ, HEAD_DIM)), OP.mult)
                        # rope+norm for k
                        k1 = kf[:nt, 0:64]
                        k2 = kf[:nt, 64:128]
                        kr = pa2.tile([P, HEAD_DIM], f32, tag="kr")
                        t1k = pa1.tile([P, 64], f32, tag="t1k")
                        t2k = pa1.tile([P, 64], f32, tag="t2k")
                        nc.vector.tensor_tensor(t1k[:nt], k1,
                                                rp_t[:nt, 4, :], OP.mult)
                        nc.vector.tensor_tensor(t2k[:nt], k2,
                                                rp_t[:nt, 5, :], OP.mult)
                        nc.vector.tensor_tensor(kr[:nt, 0:64], t1k[:nt],
                                                t2k[:nt], OP.subtract)
                        nc.vector.tensor_tensor(t1k[:nt], k1,
                                                rp_t[:nt, 6, :], OP.mult)
                        nc.vector.tensor_tensor(t2k[:nt], k2,
                                                rp_t[:nt, 7, :], OP.mult)
                        nc.vector.tensor_tensor(kr[:nt, 64:128], t1k[:nt],
                                                t2k[:nt], OP.add)
                        nc.vector.tensor_scalar_mul(kr[:nt], kr[:nt],
                                                    rq[:nt, GQ:GQ + 1])
                        # transposes into [d, n] globals
                        for h in range(GQ):
                            tp = psT.tile([P, P], f32, tag="tp")
                            nc.tensor.transpose(tp[:, :nt], qr[:nt, h, :],
                                                ident[:nt, :nt])
                            nc.vector.tensor_copy(
                                out=qkT[:, h, tok0:tok0 + nt],
                                in_=tp[:, :nt])
                        tp = psT.tile([P, P], f32, tag="tp")
                        nc.tensor.transpose(tp[:, :nt], kr[:nt],
                                            ident[:nt, :nt])
                        nc.vector.tensor_copy(out=qkT[:, GQ, tok0:tok0 + nt],
                                              in_=tp[:, :nt])
                        tpb = psT.tile([P, P], bf16, tag="tp")
                        nc.tensor.transpose(tpb[:, :nt], vf[:nt],
                                            ident_bf[:nt, :nt])
                        nc.vector.tensor_copy(out=vT_g[:, tok0:tok0 + nt],
                                              in_=tpb[:, :nt])
                        tpg = psT.tile([P, P], f32, tag="tp")
                        nc.tensor.transpose(tpg[0:GQ, :nt], gf[:nt, 0:GQ],
                                            ident[:nt, :nt])
                        nc.vector.tensor_copy(out=g_sig[0:GQ, tok0:tok0 + nt],
                                              in_=tpg[0:GQ, :nt])
                # A2: re-tile v into [m, d] blocks + gate sigmoid
                for m in range(NB):
                    tpb = psT.tile([P, P], bf16, tag="tp")
                    nc.tensor.transpose(tpb[:], vT_g[:, m * P:(m + 1) * P],
                                        ident_bf[:])
                    nc.vector.tensor_copy(out=v_all[:, m, :], in_=tpb[:])
                nc.scalar.activation(g_sig[0:GQ, :], g_sig[0:GQ, :],
                                     AF.Sigmoid)
                # engines can only address partition 0 upward, so bounce the
                # per-head sigmoid rows through DRAM for phase B row reads
                gd = gdram.tile([GQ, N_TOK], f32)
                nc.sync.dma_start(out=gd[:], in_=g_sig[0:GQ, :])

            # ---------------- phase B: attention ---------------------------
            # Softmax denominator = DVE running sum of the exp tiles + one PE
            # ones-colsum (saves 320 PE MMs and the dedicated PSUM bank).
            # pcw (proj weights) opens BEFORE the B pools so its addresses
            # don't alias them — the scheduler can then prefetch proj weights
            # during attention.
            pcw_ctx = tc.tile_pool(name="pcw", bufs=1)
            pcw = pcw_ctx.__enter__()
            with tc.tile_pool(name="pb2", bufs=2) as pb2, \
                 tc.tile_pool(name="pb3", bufs=3) as pb3, \
                 tc.tile_pool(name="dramb", bufs=2, space="DRAM") as dramb, \
                 tc.tile_pool(name="psS", bufs=2, space="PSUM") as psS, \
                 tc.tile_pool(name="psO", bufs=2, space="PSUM") as psO:
                for c in range(N_TOK // N2):
                    nsl = slice(c * N2, (c + 1) * N2)
                    for h in range(GQ):
                        o_ps = psO.tile([P, N2], f32, tag="o")
                        acc = pb2.tile([P, N2], bf16, tag="acc")
                        for m in range(NB):
                            s_ps = psS.tile([P, N2], f32, tag="s")
                            for u in range(N2 // 512):
                                nc.tensor.matmul(
                                    s_ps[:, u * 512:(u + 1) * 512],
                                    lhsT=qkT[:, GQ, m * P:(m + 1) * P],
                                    rhs=qkT[:, h, c * N2 + u * 512:
                                            c * N2 + (u + 1) * 512],
                                    start=True, stop=True)
                            pT = pb3.tile([P, N2], bf16, tag="pT")
                            nc.scalar.activation(pT[:], s_ps[:], AF.Exp,
                                                 scale=SCALE)
                            for u in range(N2 // 512):
                                usl = slice(u * 512, (u + 1) * 512)
                                nc.tensor.matmul(
                                    o_ps[:, usl], lhsT=v_all[:, m, :],
                                    rhs=pT[:, usl],
                                    start=(m == 0), stop=(m == NB - 1))
                            if m == 0:
                                nc.vector.tensor_copy(out=acc[:], in_=pT[:])
                            else:
                                nc.vector.tensor_tensor(acc[:], acc[:],
                                                        pT[:], OP.add)
                        d_ps = psS.tile([P, N2], f32, tag="s")
                        for u in range(N2 // 512):
                            usl = slice(u * 512, (u + 1) * 512)
                            nc.tensor.matmul(
                                d_ps[0:1, usl], lhsT=ones_bf[:, 0:1],
                                rhs=acc[:, usl], start=True, stop=True)
                        dinv = pb2.tile([1, N2], f32, tag="dinv")
                        nc.vector.reciprocal(dinv[:], d_ps[0:1, :])
                        sgrow = pb2.tile([1, N2], f32, tag="sgrow")
                        nc.sync.dma_start(out=sgrow[:], in_=gd[h:h + 1, nsl])
                        nc.vector.tensor_tensor(dinv[:], dinv[:],
                                                sgrow[:], OP.mult)
                        dsc = dramb.tile([1, N2], f32, tag="dsc")
                        nc.sync.dma_start(out=dsc[:], in_=dinv[:])
                        rb = pb2.tile([P, N2], f32, tag="rb")
                        nc.sync.dma_start(
                            out=rb[:], in_=dsc[0:1, :].to_broadcast((P, N2)))
                        nc.vector.tensor_tensor(oT_all[:, h, nsl], o_ps[:],
                                                rb[:], OP.mult)

            # ---------------- phase C: output projection -------------------
            with tc.tile_pool(name="pc3", bufs=3) as pc3, \
                 tc.tile_pool(name="psC", bufs=4, space="PSUM") as psC:
                HQT = HIDDEN // 4       # 1280 output cols per weight quarter
                for g in range(3):
                    wp_sb = []
                    for q in range(4):
                        wt = pcw.tile([P, GQ, HQT], bf16, tag=f"wp{q}")
                        nc.sync.dma_start(
                            out=wt[:],
                            in_=wproj[g, :, :, q * HQT:(q + 1) * HQT]
                            .rearrange("fo p h -> p fo h"))
                        wp_sb.append(wt)
                    for (c0, cn, gg) in chunks:
                        if gg != g:
                            continue
                        for ht in range(HIDDEN // P):
                            wt = wp_sb[ht * P // HQT]
                            ho = ht * P % HQT
                            po = psC.tile([P, 512], f32, tag="po")
                            for f in range(GQ):
                                nc.tensor.matmul(
                                    po[:, :cn],
                                    lhsT=wt[:, f, ho:ho + P],
                                    rhs=oT_all[:, f, c0:c0 + cn],
                                    start=(f == 0), stop=(f == GQ - 1))
                            ob = pc3.tile([P, 512], f32, tag="ob")
                            if ht % 2 == 0:
                                nc.vector.tensor_copy(out=ob[:, :cn],
                                                      in_=po[:, :cn])
                            else:
                                nc.scalar.copy(out=ob[:, :cn], in_=po[:, :cn])
                            nc.sync.dma_start(
                                out=outT[ht * P:(ht + 1) * P, c0:c0 + cn],
                                in_=ob[:, :cn])
            pcw_ctx.__exit__(None, None, None)

    return nc, tiles, xt_offs, xt_total


# ---------------------------------------------------------------------------
# host wrapper
# ---------------------------------------------------------------------------

def prepare(hidden_states, rope, pre_norm_w, qkv_w, q_norm_w, k_norm_w,
            proj_w, modality_ids):
    """Host-side layout prep. Returns (counts, perm, in_maps_fn) where
    in_maps_fn(tiles, xt_offs, xt_total) builds the per-core input maps."""
    import ml_dtypes

    bf16 = ml_dtypes.bfloat16
    x = np.asarray(hidden_states, np.float32)
    rope = np.asarray(rope, np.float32)
    pre_w = np.asarray(pre_norm_w, np.float32).reshape(NUM_MOD, HIDDEN)
    qkv_w = np.asarray(qkv_w, np.float32).reshape(NUM_MOD, QKV_OUT, HIDDEN)
    qn_w = np.asarray(q_norm_w, np.float32).reshape(NUM_MOD, HEAD_DIM)
    kn_w = np.asarray(k_norm_w, np.float32).reshape(NUM_MOD, HEAD_DIM)
    proj_w = np.asarray(proj_w, np.float32).reshape(NUM_MOD, HIDDEN, Q_SIZE)
    mids = np.asarray(modality_ids).astype(np.int64)

    perm = np.argsort(mids, kind="stable")
    counts = tuple(int((mids == g).sum()) for g in range(NUM_MOD))
    x_p = x[perm]
    rope_p = rope[perm]
    mids_p = mids[perm]

    # ---- rope coefficient tables (fold q/k-norm w+1) ----
    sin = rope_p[:, :64]
    cos = rope_p[:, 64:]
    wq = qn_w[mids_p] + 1.0                             # [N, 128]
    wk = kn_w[mids_p] + 1.0
    ropec = np.empty((N_TOK, 8, 64), np.float32)
    ropec[:, 0] = cos * wq[:, :64]
    ropec[:, 1] = sin * wq[:, 64:]
    ropec[:, 2] = sin * wq[:, :64]
    ropec[:, 3] = cos * wq[:, 64:]
    ropec[:, 4] = cos * wk[:, :64]
    ropec[:, 5] = sin * wk[:, 64:]
    ropec[:, 6] = sin * wk[:, :64]
    ropec[:, 7] = cos * wk[:, 64:]

    # ---- per-core weight slices ----
    wqkv_cores = []
    wproj_cores = []
    for c in range(NCORES):
        rows = np.concatenate([
            np.arange(c * QC, (c + 1) * QC),
            np.arange(Q_SIZE + c * HEAD_DIM, Q_SIZE + (c + 1) * HEAD_DIM),
            np.arange(Q_SIZE + KV_SIZE + c * HEAD_DIM,
                      Q_SIZE + KV_SIZE + (c + 1) * HEAD_DIM),
            np.arange(Q_SIZE + 2 * KV_SIZE + c * GQ,
                      Q_SIZE + 2 * KV_SIZE + (c + 1) * GQ),
        ])
        wc = qkv_w[:, rows, :] * (pre_w[:, None, :] + 1.0)  # [3, 901, 5120]
        wt = wc.transpose(0, 2, 1).reshape(NUM_MOD, KO, P, FC)
        wqkv_cores.append(np.ascontiguousarray(wt).astype(bf16))
        pc = proj_w[:, :, c * QC:(c + 1) * QC]              # [3, 5120, 640]
        pt = pc.transpose(0, 2, 1).reshape(NUM_MOD, GQ, P, HIDDEN)
        wproj_cores.append(np.ascontiguousarray(pt).astype(bf16))

    x_bf = x_p.astype(bf16)

    def in_maps_fn(tiles, xt_offs, xt_total):
        xt_flat = np.empty(xt_total, bf16)
        for (tok0, nt, g), off in zip(tiles, xt_offs):
            blk = x_bf[tok0:tok0 + nt]                    # [nt, 5120]
            t = blk.reshape(nt, KO, P).transpose(2, 1, 0)  # [p, ko, nt]
            xt_flat[off:off + P * KO * nt] = \
                np.ascontiguousarray(t).reshape(-1)
        return [{
            "xt": xt_flat,
            "xn": x_bf,
            "ropec": ropec,
            "wqkv": wqkv_cores[c],
            "wproj": wproj_cores[c],
        } for c in range(NCORES)]

    return counts, perm, in_maps_fn


def kernel(hidden_states, rope, pre_norm_w, qkv_w, q_norm_w, k_norm_w,
           proj_w, modality_ids):
    global LAST_EXEC_NS

    counts, perm, in_maps_fn = prepare(
        hidden_states, rope, pre_norm_w, qkv_w, q_norm_w, k_norm_w,
        proj_w, modality_ids)

    if counts not in _BUILD_CACHE:
        _install_profile_hook()
        _install_legalizer()
        _BUILD_CACHE[counts] = _build(counts)
    nc, tiles, xt_offs, xt_total = _BUILD_CACHE[counts]

    in_maps = in_maps_fn(tiles, xt_offs, xt_total)

    from concourse.bass_utils import run_bass_kernel_spmd

    trace = os.environ.get("BASSMOE_TRACE", "") == "1"
    res = run_bass_kernel_spmd(nc, in_maps, core_ids=list(range(NCORES)),
                               trace=trace)
    LAST_EXEC_NS = res.exec_time_ns

    acc = np.zeros((HIDDEN, N_TOK), np.float64)
    for c in range(NCORES):
        acc += np.asarray(res.results[c]["outT"], np.float64)
    out_p = acc.T.astype(np.float32)                    # [N, HIDDEN] permuted
    out = np.empty_like(out_p)
    out[perm] = out_p
    return out



# revision 7
# speedup vs baseline: 1.2968x; 1.2968x over previous
"""DaVinci attention (multi-modal MoE-routed attention block) on 8 Trainium2
NeuronCores.

Sharding: tensor-parallel over heads.  Each of the 8 cores owns one KV head
and its 5 GQA query heads: qkv-weight columns (640 q + 128 k + 128 v + 5 gate
per core) and proj-weight rows (640 per core) are sliced per core; the final
projection output is a partial sum reduced on the host.

Host-side prep (layout only — all FLOPs stay on device):
  * tokens are permuted so same-modality tokens are contiguous; each expert's
    GEMM then runs on its own token range (no 3x masked-dispatch waste)
  * pre-norm weight (w+1) is folded into the qkv weight columns; the
    per-token rms scale is applied on-device after the GEMM
  * q/k-norm weights (w+1) are folded into host-precomputed rope coefficient
    tables A=cos*(w1+1), B=sin*(w2+1), D=sin*(w1+1), E=cos*(w2+1)
  * weights are pre-transposed/tiled for contraction-major DMA

Schedule: phase A (rms + qkv GEMM + norms + rope) fills qkT/v_all; phase B
(attention) and phase C (output projection) are fused — C's GEMMs for the
first half of the sequence are interleaved under B's second half so the PE
array stays busy while the scalar engine runs softmax exps.  The softmax
denominator row is produced+broadcast in one gpsimd partition_all_reduce and
inverted with the fast DVE reciprocal; sigmoid gate rows are broadcast once
per head.
"""

import os
import sys
import types

import numpy as np

HIDDEN = 5120
HEAD_DIM = 128
HQ = 40
HKV = 8
NUM_MOD = 3
Q_SIZE = HQ * HEAD_DIM          # 5120
KV_SIZE = HKV * HEAD_DIM        # 1024
GATE = HQ
QKV_OUT = Q_SIZE + 2 * KV_SIZE + GATE  # 7208
EPS = 1e-6
N_TOK = 2048
P = 128
NCORES = 8
GQ = HQ // HKV                  # 5 q heads per core
QC = GQ * HEAD_DIM              # 640 q cols per core
FC = QC + 2 * HEAD_DIM + GQ     # 901 qkv out features per core
KO = HIDDEN // P                # 40 contraction chunks
NB = N_TOK // P                 # 16 token blocks of 128 (attention tiling)
N2 = 1024                       # attention free-dim chunk
HT = HIDDEN // P                # 40 proj output row-blocks
SCALE = 1.0 / float(np.sqrt(HEAD_DIM))

LAST_EXEC_NS = None             # filled when BASSMOE_TRACE=1


# ---------------------------------------------------------------------------
# axon NTFF profiling hook (needed only when tracing) + BIR sync legalizer
# ---------------------------------------------------------------------------

def _install_profile_hook():
    if "antenv.axon_hooks" in sys.modules:
        return
    mod = types.ModuleType("antenv.axon_hooks")
    _h = [None]
    mod.set_axon_ntff_profile_hook = lambda h: _h.__setitem__(0, h)
    mod.get_axon_ntff_profile_hook = lambda: _h[0]
    import antenv

    antenv.axon_hooks = mod
    sys.modules["antenv.axon_hooks"] = mod
    try:
        from trn_agent_boot.trn_boot import _ntff_profile_via_ctypes

        mod.set_axon_ntff_profile_hook(
            _ntff_profile_via_ctypes("/opt/axon/libaxon_pjrt.so")
        )
    except Exception:
        pass


def _legalize_sync(bir_json):
    """This walrus build accepts a single sync wait/update per instruction.
    Move extra waits onto preceding same-engine NoOps (the engine stalls
    before dispatch either way) and extra updates onto trailing NoOps."""
    import json

    data = json.loads(bir_json)
    for fn in data["functions"]:
        for blk in fn["blocks"]:
            out = []
            for ins in blk["instructions"]:
                si = ins.get("sync_info")
                waits = si.get("on_wait", []) if si else []
                upds = si.get("on_update", []) if si else []
                if len(waits) > 1:
                    for i, w in enumerate(waits[:-1]):
                        out.append({
                            "debug": ins.get("debug", 0),
                            "engine": ins["engine"],
                            "ins": [], "is_reset_sema": False,
                            "name": f"{ins['name']}-lw{i}",
                            "opcode": "NoOp", "outs": [],
                            "sync_info": {"on_update": [], "on_wait": [w]},
                        })
                    si["on_wait"] = [waits[-1]]
                out.append(ins)
                if len(upds) > 1:
                    if ins["opcode"] in ("DMACopy", "DMATranspose"):
                        raise AssertionError(
                            f"DMA instruction {ins['name']} has multiple updates")
                    for i, u in enumerate(upds[1:]):
                        out.append({
                            "debug": ins.get("debug", 0),
                            "engine": ins["engine"],
                            "ins": [], "is_reset_sema": False,
                            "name": f"{ins['name']}-lu{i}",
                            "opcode": "NoOp", "outs": [],
                            "sync_info": {"on_update": [u], "on_wait": []},
                        })
                    si["on_update"] = [upds[0]]
            blk["instructions"] = out
    return json.dumps(data).encode()


def _install_legalizer():
    from concourse import bass2jax, bass_utils

    if getattr(bass2jax, "_sync_legalize_installed", False):
        return
    orig = bass_utils.compile_bir_kernel

    def wrapped(bir_json, tmpdir, neff_name="file.neff"):
        return orig(_legalize_sync(bir_json), tmpdir, neff_name)

    bass2jax.compile_bir_kernel = wrapped
    bass_utils.compile_bir_kernel = wrapped
    bass2jax._sync_legalize_installed = True


# ---------------------------------------------------------------------------
# device program
# ---------------------------------------------------------------------------

_BUILD_CACHE = {}


def _build(counts):
    import concourse.bass as bass
    import concourse.tile as tile
    from concourse import mybir, bass_isa
    from concourse.masks import make_identity

    f32 = mybir.dt.float32
    bf16 = mybir.dt.bfloat16
    AF = mybir.ActivationFunctionType
    OP = mybir.AluOpType

    n0, n1, n2 = counts
    starts = [0, n0, n0 + n1, 2048]
    # group-chunked qkv tiles (tok0, nt, g)
    tiles = []
    for g in range(3):
        t0, t1 = starts[g], starts[g + 1]
        for a in range(t0, t1, P):
            tiles.append((a, min(P, t1 - a), g))
    # packed-xt flat offsets per tile
    xt_offs = []
    off = 0
    for (a, nt, g) in tiles:
        xt_offs.append(off)
        off += P * KO * nt
    xt_total = off

    # proj work: per B-segment (N2 tokens), per group, the contiguous token
    # ranges (split <=512 for PSUM); each (seg, g) then sweeps 40 ht blocks
    csegs = []                       # (seg, g, base, [(a, b), ...])
    for seg in range(N_TOK // N2):
        lo, hi = seg * N2, (seg + 1) * N2
        for g in range(3):
            a0, b0 = max(lo, starts[g]), min(hi, starts[g + 1])
            if a0 < b0:
                rngs = [(a, min(a + 512, b0)) for a in range(a0, b0, 512)]
                csegs.append((seg, g, a0, rngs))
    cunits = [(ci, ht) for ci, cs in enumerate(csegs) for ht in range(HT)]
    cunits_s0 = [(ci, ht) for (ci, ht) in cunits if csegs[ci][0] == 0]
    cunits_s1 = [(ci, ht) for (ci, ht) in cunits if csegs[ci][0] == 1]
    ob_max = max(cs[3][-1][1] - cs[2] for cs in csegs)

    nc = bass.Bass()
    xt = nc.dram_tensor("xt", (xt_total,), bf16, kind="ExternalInput")
    xn = nc.dram_tensor("xn", (N_TOK, HIDDEN), bf16, kind="ExternalInput")
    ropec = nc.dram_tensor("ropec", (N_TOK, 8, 64), f32, kind="ExternalInput")
    wqkv = nc.dram_tensor("wqkv", (NUM_MOD, KO, P, FC), bf16, kind="ExternalInput")
    wproj = nc.dram_tensor("wproj", (NUM_MOD, HT, GQ, P, P), bf16,
                           kind="ExternalInput")
    outT = nc.dram_tensor("outT", (HIDDEN, N_TOK), bf16, kind="ExternalOutput")

    with tile.TileContext(nc) as tc:
        with tc.tile_pool(name="cst", bufs=1) as cst, \
             tc.tile_pool(name="gdram", bufs=1, space="DRAM") as gdram, \
             tc.tile_pool(name="glob", bufs=1) as glob:
            ident = cst.tile([P, P], f32)
            make_identity(nc, ident)
            ident_bf = cst.tile([P, P], bf16)
            make_identity(nc, ident_bf)
            eps_t = cst.tile([P, 1], f32)
            nc.vector.memset(eps_t, EPS)
            ones_bf = cst.tile([P, P], bf16)
            nc.vector.memset(ones_bf, 1.0)

            # persistent activations
            qkT = glob.tile([P, 6, N_TOK], bf16)     # [d, head(0-4=q,5=k), n]
            v_all = glob.tile([P, NB, P], bf16)      # [n%128, n//128, d]
            oT_all = glob.tile([P, GQ, N_TOK], bf16)  # [d, head, n]
            gd = gdram.tile([GQ, N_TOK], f32)        # sigmoid(gate) rows

            # ---------------- phase A: rms + qkv GEMM + norms + rope ------
            with tc.tile_pool(name="paw", bufs=1) as paw, \
                 tc.tile_pool(name="pa2", bufs=2) as pa2, \
                 tc.tile_pool(name="pa1", bufs=1) as pa1, \
                 tc.tile_pool(name="paG", bufs=1) as paG, \
                 tc.tile_pool(name="psA", bufs=6, space="PSUM") as psA, \
                 tc.tile_pool(name="psT", bufs=2, space="PSUM") as psT:
                g_sig = paG.tile([GQ, N_TOK], f32)
                KQ = KO // 4            # 10 ko per weight quarter
                for g in range(3):
                    # quarter the group weight so the next group's quarters
                    # stream in under this group's matmuls
                    wq_sb = []
                    for q in range(4):
                        wt = paw.tile([P, KQ, FC], bf16, tag=f"wq{q}")
                        nc.sync.dma_start(
                            out=wt[:],
                            in_=wqkv[g, q * KQ:(q + 1) * KQ]
                            .rearrange("ko p f -> p ko f"))
                        wq_sb.append(wt)
                    for ti, (tok0, nt, gg) in enumerate(tiles):
                        if gg != g:
                            continue
                        xt_t = pa2.tile([P, KO, P], bf16, tag="xt")
                        nc.sync.dma_start(
                            out=xt_t[:, :, :nt],
                            in_=xt[xt_offs[ti]:xt_offs[ti] + P * KO * nt]
                            .rearrange("(p ko j) -> p ko j", p=P, ko=KO))
                        xn_t = pa1.tile([P, HIDDEN], bf16, tag="xn")
                        nc.sync.dma_start(out=xn_t[:nt],
                                          in_=xn[tok0:tok0 + nt])
                        rp_t = pa2.tile([P, 8, 64], f32, tag="rp")
                        nc.sync.dma_start(out=rp_t[:nt],
                                          in_=ropec[tok0:tok0 + nt])
                        # pre-norm rms (from raw x): sum(x^2) via ScalarE
                        # Square+accum, then sqrt(acc/H + eps), reciprocal
                        ssq = pa2.tile([P, 1], f32, tag="ssq")
                        nc.scalar.activation(out=xn_t[:nt], in_=xn_t[:nt],
                                             func=AF.Square,
                                             accum_out=ssq[:nt])
                        srt = pa2.tile([P, 1], f32, tag="srt")
                        nc.scalar.activation(srt[:nt], ssq[:nt], AF.Sqrt,
                                             scale=1.0 / HIDDEN,
                                             bias=eps_t[:nt])
                        rinv = pa2.tile([P, 1], f32, tag="rinv")
                        nc.vector.reciprocal(rinv[:nt], srt[:nt])
                        # qkv GEMM: psum [tokens, features]
                        ps_a = psA.tile([P, 512], f32, tag="ps512")
                        ps_b = psA.tile([P, 512], f32, tag="ps512")
                        for ko in range(KO):
                            wt = wq_sb[ko // KQ]
                            kq = ko % KQ
                            nc.tensor.matmul(
                                ps_a[:nt, :],
                                lhsT=xt_t[:, ko, :nt],
                                rhs=wt[:, kq, 0:512],
                                start=(ko == 0), stop=(ko == KO - 1))
                            nc.tensor.matmul(
                                ps_b[:nt, 0:FC - 512],
                                lhsT=xt_t[:, ko, :nt],
                                rhs=wt[:, kq, 512:FC],
                                start=(ko == 0), stop=(ko == KO - 1))
                        # evacuate with rms scale
                        qf = pa1.tile([P, GQ, HEAD_DIM], f32, tag="qf")
                        kf = pa1.tile([P, HEAD_DIM], f32, tag="kf")
                        vf = pa1.tile([P, HEAD_DIM], bf16, tag="vf")
                        gf = pa1.tile([P, 8], f32, tag="gf")
                        nc.vector.tensor_scalar_mul(
                            qf[:nt, 0:4, :], ps_a[:nt, :], rinv[:nt])
                        nc.vector.tensor_scalar_mul(
                            qf[:nt, 4, :], ps_b[:nt, 0:128], rinv[:nt])
                        nc.vector.tensor_scalar_mul(
                            kf[:nt, :], ps_b[:nt, 128:256], rinv[:nt])
                        nc.vector.tensor_scalar_mul(
                            vf[:nt, :], ps_b[:nt, 256:384], rinv[:nt])
                        nc.vector.tensor_scalar_mul(
                            gf[:nt, 0:GQ], ps_b[:nt, 384:389], rinv[:nt])
                        # v: straight into [n%128, n//128, d] via sbuf dma
                        o0, b0 = tok0 % P, tok0 // P
                        k1 = min(nt, P - o0)
                        nc.sync.dma_start(out=v_all[o0:o0 + k1, b0, :],
                                          in_=vf[0:k1, :])
                        if nt > k1:
                            nc.sync.dma_start(out=v_all[0:nt - k1, b0 + 1, :],
                                              in_=vf[k1:nt, :])
                        # q/k rms over head_dim (Square+accum per head)
                        sq = pa2.tile([P, 8], f32, tag="sq")
                        junk = pa1.tile([P, HEAD_DIM], f32, tag="junk")
                        for h in range(GQ):
                            nc.scalar.activation(
                                out=junk[:nt], in_=qf[:nt, h, :],
                                func=AF.Square,
                                accum_out=sq[:nt, h:h + 1])
                        nc.scalar.activation(
                            out=junk[:nt], in_=kf[:nt], func=AF.Square,
                            accum_out=sq[:nt, GQ:GQ + 1])
                        sqs = pa2.tile([P, 8], f32, tag="sqs")
                        nc.scalar.activation(sqs[:nt, 0:6], sq[:nt, 0:6],
                                             AF.Sqrt, scale=1.0 / HEAD_DIM,
                                             bias=eps_t[:nt])
                        rq = pa2.tile([P, 8], f32, tag="rq")
                        nc.vector.reciprocal(rq[:nt, 0:6], sqs[:nt, 0:6])
                        # rope+norm for q (coeff tables already fold w+1)
                        q1 = qf[:nt, :, 0:64]
                        q2 = qf[:nt, :, 64:128]
                        t1 = pa1.tile([P, GQ, 64], f32, tag="t1")
                        t2 = pa1.tile([P, GQ, 64], f32, tag="t2")
                        qr = pa2.tile([P, GQ, HEAD_DIM], f32, tag="qr")
                        qrb = pa2.tile([P, GQ, HEAD_DIM], bf16, tag="qrb")

                        def bc(i):
                            return rp_t[:nt, i:i + 1, :].to_broadcast(
                                (nt, GQ, 64))

                        nc.vector.tensor_tensor(t1[:nt], q1, bc(0), OP.mult)
                        nc.vector.tensor_tensor(t2[:nt], q2, bc(1), OP.mult)
                        nc.vector.tensor_tensor(qr[:nt, :, 0:64], t1[:nt],
                                                t2[:nt], OP.subtract)
                        nc.vector.tensor_tensor(t1[:nt], q1, bc(2), OP.mult)
                        nc.vector.tensor_tensor(t2[:nt], q2, bc(3), OP.mult)
                        nc.vector.tensor_tensor(qr[:nt, :, 64:128], t1[:nt],
                                                t2[:nt], OP.add)
                        nc.vector.tensor_tensor(
                            qrb[:nt], qr[:nt],
                            rq[:nt, 0:GQ, None].to_broadcast(
                                (nt, GQ, HEAD_DIM)), OP.mult)
                        # rope+norm for k
                        k1f = kf[:nt, 0:64]
                        k2f = kf[:nt, 64:128]
                        kr = pa2.tile([P, HEAD_DIM], f32, tag="kr")
                        krb = pa2.tile([P, HEAD_DIM], bf16, tag="krb")
                        t1k = pa1.tile([P, 64], f32, tag="t1k")
                        t2k = pa1.tile([P, 64], f32, tag="t2k")
                        nc.vector.tensor_tensor(t1k[:nt], k1f,
                                                rp_t[:nt, 4, :], OP.mult)
                        nc.vector.tensor_tensor(t2k[:nt], k2f,
                                                rp_t[:nt, 5, :], OP.mult)
                        nc.vector.tensor_tensor(kr[:nt, 0:64], t1k[:nt],
                                                t2k[:nt], OP.subtract)
                        nc.vector.tensor_tensor(t1k[:nt], k1f,
                                                rp_t[:nt, 6, :], OP.mult)
                        nc.vector.tensor_tensor(t2k[:nt], k2f,
                                                rp_t[:nt, 7, :], OP.mult)
                        nc.vector.tensor_tensor(kr[:nt, 64:128], t1k[:nt],
                                                t2k[:nt], OP.add)
                        nc.vector.tensor_scalar_mul(krb[:nt], kr[:nt],
                                                    rq[:nt, GQ:GQ + 1])
                        # transposes into [d, n] globals (bf16 streams 4x
                        # faster through the PE than f32)
                        for h in range(GQ):
                            tp = psT.tile([P, P], bf16, tag="tp")
                            nc.tensor.transpose(tp[:, :nt], qrb[:nt, h, :],
                                                ident_bf[:nt, :nt])
                            nc.vector.tensor_copy(
                                out=qkT[:, h, tok0:tok0 + nt],
                                in_=tp[:, :nt])
                        tp = psT.tile([P, P], bf16, tag="tp")
                        nc.tensor.transpose(tp[:, :nt], krb[:nt],
                                            ident_bf[:nt, :nt])
                        nc.vector.tensor_copy(out=qkT[:, GQ, tok0:tok0 + nt],
                                              in_=tp[:, :nt])
                        tpg = psT.tile([P, P], f32, tag="tp")
                        nc.tensor.transpose(tpg[0:GQ, :nt], gf[:nt, 0:GQ],
                                            ident[:nt, :nt])
                        nc.vector.tensor_copy(out=g_sig[0:GQ, tok0:tok0 + nt],
                                              in_=tpg[0:GQ, :nt])
                # gate rows -> ln(sigmoid) -> DRAM (per-head broadcast reads
                # them back in phase B; engines only address partition 0 up)
                nc.scalar.activation(g_sig[0:GQ, :], g_sig[0:GQ, :],
                                     AF.Sigmoid)
                nc.scalar.activation(g_sig[0:GQ, :], g_sig[0:GQ, :], AF.Ln)
                nc.sync.dma_start(out=gd[:], in_=g_sig[0:GQ, :])

            # ---------------- phase B+C: attention fused with projection ---
            with tc.tile_pool(name="pb", bufs=1) as pb, \
                 tc.tile_pool(name="pcw", bufs=6) as pcw, \
                 tc.tile_pool(name="pco", bufs=2) as pco, \
                 tc.tile_pool(name="psS", bufs=2, space="PSUM") as psS, \
                 tc.tile_pool(name="psO", bufs=1, space="PSUM") as psO, \
                 tc.tile_pool(name="psC", bufs=2, space="PSUM") as psC:
                # per-head ln(sigmoid) gate rows broadcast to all partitions
                lsig_b = []
                for h in range(GQ):
                    sb = pb.tile([P, N_TOK], f32, tag=f"sb{h}", name=f"sb{h}")
                    nc.sync.dma_start(
                        out=sb[:], in_=gd[h:h + 1, :].to_broadcast((P, N_TOK)))
                    lsig_b.append(sb)

                # ---- proj work units (streamed weights, batched output) ---
                wp_tiles = {}

                def c_prefetch(k, units):
                    if k < len(units) and k not in wp_tiles:
                        ci, ht = units[k]
                        g = csegs[ci][1]
                        wt = pcw.tile([P, GQ, P], bf16, tag="wp", name="wt")
                        nc.sync.dma_start(
                            out=wt[:],
                            in_=wproj[g, ht].rearrange("f d h -> d f h"))
                        wp_tiles[k] = wt

                ob_cur = [None, 0]      # tile, base token

                def emit_c_unit(k, units):
                    ci, ht = units[k]
                    seg, g, base, rngs = csegs[ci]
                    for kk in range(k + 1, k + 5):
                        c_prefetch(kk, units)
                    wt = wp_tiles.pop(k)
                    if ht % 4 == 0:
                        ob_cur[0] = pco.tile([P, 4, ob_max], bf16, tag="ob",
                                             name="ob")
                        ob_cur[1] = base
                    ob = ob_cur[0]
                    for (a, b) in rngs:
                        po = psC.tile([P, 512], f32, tag="po", name="po")
                        for f in range(GQ):
                            nc.tensor.matmul(
                                po[:, :b - a],
                                lhsT=wt[:, f, :],
                                rhs=oT_all[:, f, a:b],
                                start=(f == 0), stop=(f == GQ - 1))
                        dst = ob[:, ht % 4, a - base:b - base]
                        if ht % 2 == 0:
                            nc.vector.tensor_copy(out=dst, in_=po[:, :b - a])
                        else:
                            nc.scalar.copy(out=dst, in_=po[:, :b - a])
                    if ht % 4 == 3:
                        ctot = rngs[-1][1] - base
                        nc.sync.dma_start(
                            out=outT[(ht - 3) * P:(ht + 1) * P,
                                     base:base + ctot]
                            .rearrange("(o p) n -> p o n", p=P),
                            in_=ob[:, :, :ctot])

                # ---- attention for one (c, h) --------------------------
                def emit_b(c, h, inject):
                    nsl = slice(c * N2, (c + 1) * N2)
                    o_ps = psO.tile([P, N2], f32, tag="o", name="o_ps")
                    acc = pb.tile([P, N2], bf16, tag="acc", bufs=2,
                                  name="acc")
                    for m in range(NB):
                        s_ps = psS.tile([P, N2], f32, tag="s", name="s_ps")
                        for u in range(N2 // 512):
                            nc.tensor.matmul(
                                s_ps[:, u * 512:(u + 1) * 512],
                                lhsT=qkT[:, GQ, m * P:(m + 1) * P],
                                rhs=qkT[:, h, c * N2 + u * 512:
                                        c * N2 + (u + 1) * 512],
                                start=True, stop=True)
                        pT = pb.tile([P, N2], bf16, tag="pT", bufs=3,
                                     name="pT")
                        nc.scalar.activation(pT[:], s_ps[:], AF.Exp,
                                             scale=SCALE)
                        for u in range(N2 // 512):
                            usl = slice(u * 512, (u + 1) * 512)
                            nc.tensor.matmul(
                                o_ps[:, usl], lhsT=v_all[:, m, :],
                                rhs=pT[:, usl],
                                start=(m == 0), stop=(m == NB - 1))
                        if m == 0:
                            nc.vector.tensor_copy(out=acc[:], in_=pT[:])
                        else:
                            nc.vector.tensor_tensor(acc[:], acc[:],
                                                    pT[:], OP.add)
                        if inject is not None:
                            inject()
                    # denominator: ones-matmul column-sums acc; the 128-wide
                    # ones stationary broadcasts the row to every partition.
                    # sig/den = exp(-(ln den - ln sig)) avoids the slow DVE
                    # reciprocal; exp+ln live in one scalar table set.
                    den_ps = psS.tile([P, N2], f32, tag="s", name="den_ps")
                    for u in range(N2 // 512):
                        usl = slice(u * 512, (u + 1) * 512)
                        nc.tensor.matmul(den_ps[:, usl], lhsT=ones_bf[:],
                                         rhs=acc[:, usl],
                                         start=True, stop=True)
                    lt = pb.tile([P, N2], f32, tag="lt", bufs=2, name="lt")
                    nc.scalar.activation(lt[:], den_ps[:], AF.Ln)
                    nc.vector.tensor_tensor(lt[:], lt[:],
                                            lsig_b[h][:, nsl], OP.subtract)
                    dg = pb.tile([P, N2], bf16, tag="dg", bufs=2, name="dg")
                    nc.scalar.activation(dg[:], lt[:], AF.Exp, scale=-1.0)
                    nc.vector.tensor_tensor(oT_all[:, h, nsl], o_ps[:],
                                            dg[:], OP.mult)

                # ---- fused schedule ------------------------------------
                for h in range(GQ):
                    emit_b(0, h, None)
                for k in range(4):
                    c_prefetch(k, cunits_s0)
                ctr = [0]

                def inject_s0():
                    if ctr[0] < len(cunits_s0):
                        emit_c_unit(ctr[0], cunits_s0)
                        ctr[0] += 1

                for h in range(GQ):
                    emit_b(1, h, inject_s0)
                while ctr[0] < len(cunits_s0):
                    inject_s0()
                for k in range(4):
                    c_prefetch(k, cunits_s1)
                for k in range(len(cunits_s1)):
                    emit_c_unit(k, cunits_s1)

    return nc, tiles, xt_offs, xt_total


# ---------------------------------------------------------------------------
# host wrapper
# ---------------------------------------------------------------------------

def prepare(hidden_states, rope, pre_norm_w, qkv_w, q_norm_w, k_norm_w,
            proj_w, modality_ids):
    """Host-side layout prep. Returns (counts, perm, in_maps_fn) where
    in_maps_fn(tiles, xt_offs, xt_total) builds the per-core input maps."""
    import ml_dtypes

    bf16 = ml_dtypes.bfloat16
    x = np.asarray(hidden_states, np.float32)
    rope = np.asarray(rope, np.float32)
    pre_w = np.asarray(pre_norm_w, np.float32).reshape(NUM_MOD, HIDDEN)
    qkv_w = np.asarray(qkv_w, np.float32).reshape(NUM_MOD, QKV_OUT, HIDDEN)
    qn_w = np.asarray(q_norm_w, np.float32).reshape(NUM_MOD, HEAD_DIM)
    kn_w = np.asarray(k_norm_w, np.float32).reshape(NUM_MOD, HEAD_DIM)
    proj_w = np.asarray(proj_w, np.float32).reshape(NUM_MOD, HIDDEN, Q_SIZE)
    mids = np.asarray(modality_ids).astype(np.int64)

    perm = np.argsort(mids, kind="stable")
    counts = tuple(int((mids == g).sum()) for g in range(NUM_MOD))
    x_p = x[perm]
    rope_p = rope[perm]
    mids_p = mids[perm]

    # ---- rope coefficient tables (fold q/k-norm w+1) ----
    sin = rope_p[:, :64]
    cos = rope_p[:, 64:]
    wq = qn_w[mids_p] + 1.0                             # [N, 128]
    wk = kn_w[mids_p] + 1.0
    ropec = np.empty((N_TOK, 8, 64), np.float32)
    ropec[:, 0] = cos * wq[:, :64]
    ropec[:, 1] = sin * wq[:, 64:]
    ropec[:, 2] = sin * wq[:, :64]
    ropec[:, 3] = cos * wq[:, 64:]
    ropec[:, 4] = cos * wk[:, :64]
    ropec[:, 5] = sin * wk[:, 64:]
    ropec[:, 6] = sin * wk[:, :64]
    ropec[:, 7] = cos * wk[:, 64:]

    # ---- per-core weight slices ----
    wqkv_cores = []
    wproj_cores = []
    for c in range(NCORES):
        rows = np.concatenate([
            np.arange(c * QC, (c + 1) * QC),
            np.arange(Q_SIZE + c * HEAD_DIM, Q_SIZE + (c + 1) * HEAD_DIM),
            np.arange(Q_SIZE + KV_SIZE + c * HEAD_DIM,
                      Q_SIZE + KV_SIZE + (c + 1) * HEAD_DIM),
            np.arange(Q_SIZE + 2 * KV_SIZE + c * GQ,
                      Q_SIZE + 2 * KV_SIZE + (c + 1) * GQ),
        ])
        wc = qkv_w[:, rows, :] * (pre_w[:, None, :] + 1.0)  # [3, 901, 5120]
        wt = wc.transpose(0, 2, 1).reshape(NUM_MOD, KO, P, FC)
        wqkv_cores.append(np.ascontiguousarray(wt).astype(bf16))
        # proj slice: [3, 40, 5, 128(d), 128(hcol)]
        pc = proj_w[:, :, c * QC:(c + 1) * QC]              # [3, 5120, 640]
        pt = pc.reshape(NUM_MOD, HT, P, GQ, HEAD_DIM).transpose(0, 1, 3, 4, 2)
        wproj_cores.append(np.ascontiguousarray(pt).astype(bf16))

    x_bf = x_p.astype(bf16)

    def in_maps_fn(tiles, xt_offs, xt_total):
        xt_flat = np.empty(xt_total, bf16)
        for (tok0, nt, g), off in zip(tiles, xt_offs):
            blk = x_bf[tok0:tok0 + nt]                    # [nt, 5120]
            t = blk.reshape(nt, KO, P).transpose(2, 1, 0)  # [p, ko, nt]
            xt_flat[off:off + P * KO * nt] = \
                np.ascontiguousarray(t).reshape(-1)
        return [{
            "xt": xt_flat,
            "xn": x_bf,
            "ropec": ropec,
            "wqkv": wqkv_cores[c],
            "wproj": wproj_cores[c],
        } for c in range(NCORES)]

    return counts, perm, in_maps_fn


def kernel(hidden_states, rope, pre_norm_w, qkv_w, q_norm_w, k_norm_w,
           proj_w, modality_ids):
    global LAST_EXEC_NS

    counts, perm, in_maps_fn = prepare(
        hidden_states, rope, pre_norm_w, qkv_w, q_norm_w, k_norm_w,
        proj_w, modality_ids)

    if counts not in _BUILD_CACHE:
        _install_profile_hook()
        _install_legalizer()
        _BUILD_CACHE[counts] = _build(counts)
    nc, tiles, xt_offs, xt_total = _BUILD_CACHE[counts]

    in_maps = in_maps_fn(tiles, xt_offs, xt_total)

    from concourse.bass_utils import run_bass_kernel_spmd

    trace = os.environ.get("BASSMOE_TRACE", "") == "1"
    res = run_bass_kernel_spmd(nc, in_maps, core_ids=list(range(NCORES)),
                               trace=trace)
    LAST_EXEC_NS = res.exec_time_ns

    acc = np.zeros((HIDDEN, N_TOK), np.float64)
    for c in range(NCORES):
        acc += np.asarray(res.results[c]["outT"], np.float64)
    out_p = acc.T.astype(np.float32)                    # [N, HIDDEN] permuted
    out = np.empty_like(out_p)
    out[perm] = out_p
    return out


# revision 14
# speedup vs baseline: 1.3130x; 1.0125x over previous
"""DaVinci attention (multi-modal MoE-routed attention block) on 8 Trainium2
NeuronCores.

Sharding: tensor-parallel over heads.  Each of the 8 cores owns one KV head
and its 5 GQA query heads: qkv-weight columns (640 q + 128 k + 128 v + 5 gate
per core) and proj-weight rows (640 per core) are sliced per core; the final
projection output is a partial sum reduced on the host.

Host-side prep (layout only — all FLOPs stay on device):
  * tokens are permuted so same-modality tokens are contiguous; each expert's
    GEMM then runs on its own token range (no 3x masked-dispatch waste)
  * pre-norm weight (w+1) is folded into the qkv weight columns; the
    per-token rms scale is applied on-device after the GEMM
  * q/k-norm weights (w+1) are folded into host-precomputed rope coefficient
    tables A=cos*(w1+1), B=sin*(w2+1), D=sin*(w1+1), E=cos*(w2+1)
  * weights are pre-transposed/tiled for contraction-major DMA

Schedule: phase A (rms + qkv GEMM + norms + rope) fills qkT/v_all; phase B
(attention) and phase C (output projection) are fused — C's GEMMs for the
first half of the sequence are interleaved under B's second half so the PE
array stays busy while the scalar engine runs softmax exps.  The softmax
denominator row is produced+broadcast in one gpsimd partition_all_reduce and
inverted with the fast DVE reciprocal; sigmoid gate rows are broadcast once
per head.
"""

import os
import sys
import types

import numpy as np

HIDDEN = 5120
HEAD_DIM = 128
HQ = 40
HKV = 8
NUM_MOD = 3
Q_SIZE = HQ * HEAD_DIM          # 5120
KV_SIZE = HKV * HEAD_DIM        # 1024
GATE = HQ
QKV_OUT = Q_SIZE + 2 * KV_SIZE + GATE  # 7208
EPS = 1e-6
N_TOK = 2048
P = 128
NCORES = 8
GQ = HQ // HKV                  # 5 q heads per core
QC = GQ * HEAD_DIM              # 640 q cols per core
FC = QC + 2 * HEAD_DIM + GQ     # 901 qkv out features per core
KO = HIDDEN // P                # 40 contraction chunks
NB = N_TOK // P                 # 16 token blocks of 128 (attention tiling)
N2 = 1024                       # attention free-dim chunk
HT = HIDDEN // P                # 40 proj output row-blocks
SCALE = 1.0 / float(np.sqrt(HEAD_DIM))

LAST_EXEC_NS = None             # filled when BASSMOE_TRACE=1


# ---------------------------------------------------------------------------
# axon NTFF profiling hook (needed only when tracing) + BIR sync legalizer
# ---------------------------------------------------------------------------

def _install_profile_hook():
    if "antenv.axon_hooks" in sys.modules:
        return
    mod = types.ModuleType("antenv.axon_hooks")
    _h = [None]
    mod.set_axon_ntff_profile_hook = lambda h: _h.__setitem__(0, h)
    mod.get_axon_ntff_profile_hook = lambda: _h[0]
    import antenv

    antenv.axon_hooks = mod
    sys.modules["antenv.axon_hooks"] = mod
    try:
        from trn_agent_boot.trn_boot import _ntff_profile_via_ctypes

        mod.set_axon_ntff_profile_hook(
            _ntff_profile_via_ctypes("/opt/axon/libaxon_pjrt.so")
        )
    except Exception:
        pass


def _legalize_sync(bir_json):
    """This walrus build accepts a single sync wait/update per instruction.
    Move extra waits onto preceding same-engine NoOps (the engine stalls
    before dispatch either way) and extra updates onto trailing NoOps."""
    import json

    data = json.loads(bir_json)
    for fn in data["functions"]:
        for blk in fn["blocks"]:
            out = []
            for ins in blk["instructions"]:
                si = ins.get("sync_info")
                waits = si.get("on_wait", []) if si else []
                upds = si.get("on_update", []) if si else []
                if len(waits) > 1:
                    for i, w in enumerate(waits[:-1]):
                        out.append({
                            "debug": ins.get("debug", 0),
                            "engine": ins["engine"],
                            "ins": [], "is_reset_sema": False,
                            "name": f"{ins['name']}-lw{i}",
                            "opcode": "NoOp", "outs": [],
                            "sync_info": {"on_update": [], "on_wait": [w]},
                        })
                    si["on_wait"] = [waits[-1]]
                out.append(ins)
                if len(upds) > 1:
                    if ins["opcode"] in ("DMACopy", "DMATranspose"):
                        raise AssertionError(
                            f"DMA instruction {ins['name']} has multiple updates")
                    for i, u in enumerate(upds[1:]):
                        out.append({
                            "debug": ins.get("debug", 0),
                            "engine": ins["engine"],
                            "ins": [], "is_reset_sema": False,
                            "name": f"{ins['name']}-lu{i}",
                            "opcode": "NoOp", "outs": [],
                            "sync_info": {"on_update": [u], "on_wait": []},
                        })
                    si["on_update"] = [upds[0]]
            blk["instructions"] = out
    return json.dumps(data).encode()


def _install_legalizer():
    from concourse import bass2jax, bass_utils

    if getattr(bass2jax, "_sync_legalize_installed", False):
        return
    orig = bass_utils.compile_bir_kernel

    def wrapped(bir_json, tmpdir, neff_name="file.neff"):
        return orig(_legalize_sync(bir_json), tmpdir, neff_name)

    bass2jax.compile_bir_kernel = wrapped
    bass_utils.compile_bir_kernel = wrapped
    bass2jax._sync_legalize_installed = True


# ---------------------------------------------------------------------------
# device program
# ---------------------------------------------------------------------------

_BUILD_CACHE = {}


def _build(counts):
    import concourse.bass as bass
    import concourse.tile as tile
    from concourse import mybir, bass_isa
    from concourse.masks import make_identity

    f32 = mybir.dt.float32
    bf16 = mybir.dt.bfloat16
    AF = mybir.ActivationFunctionType
    OP = mybir.AluOpType

    n0, n1, n2 = counts
    starts = [0, n0, n0 + n1, 2048]
    # group-chunked qkv tiles (tok0, nt, g)
    tiles = []
    for g in range(3):
        t0, t1 = starts[g], starts[g + 1]
        for a in range(t0, t1, P):
            tiles.append((a, min(P, t1 - a), g))
    # packed-xt flat offsets per tile
    xt_offs = []
    off = 0
    for (a, nt, g) in tiles:
        xt_offs.append(off)
        off += P * KO * nt
    xt_total = off

    # proj work: per B-segment (N2 tokens), per group, the contiguous token
    # ranges (split <=512 for PSUM); each (seg, g) then sweeps 40 ht blocks
    csegs = []                       # (seg, g, base, [(a, b), ...])
    for seg in range(N_TOK // N2):
        lo, hi = seg * N2, (seg + 1) * N2
        for g in range(3):
            a0, b0 = max(lo, starts[g]), min(hi, starts[g + 1])
            if a0 < b0:
                rngs = [(a, min(a + 512, b0)) for a in range(a0, b0, 512)]
                csegs.append((seg, g, a0, rngs))
    cunits = [(ci, ht) for ci, cs in enumerate(csegs) for ht in range(HT)]
    cunits_s0 = [(ci, ht) for (ci, ht) in cunits if csegs[ci][0] == 0]
    cunits_s1 = [(ci, ht) for (ci, ht) in cunits if csegs[ci][0] == 1]
    ob_max = max(cs[3][-1][1] - cs[2] for cs in csegs)

    nc = bass.Bass()
    xt = nc.dram_tensor("xt", (xt_total,), bf16, kind="ExternalInput")
    xn = nc.dram_tensor("xn", (N_TOK, HIDDEN), bf16, kind="ExternalInput")
    ropec = nc.dram_tensor("ropec", (N_TOK, 8, 64), f32, kind="ExternalInput")
    wqkv = nc.dram_tensor("wqkv", (NUM_MOD, KO, P, FC), bf16, kind="ExternalInput")
    wproj = nc.dram_tensor("wproj", (NUM_MOD, HT, GQ, P, P), bf16,
                           kind="ExternalInput")
    outT = nc.dram_tensor("outT", (HIDDEN, N_TOK), bf16, kind="ExternalOutput")

    with tile.TileContext(nc) as tc:
        with tc.tile_pool(name="cst", bufs=1) as cst, \
             tc.tile_pool(name="gdram", bufs=1, space="DRAM") as gdram, \
             tc.tile_pool(name="glob", bufs=1) as glob:
            ident = cst.tile([P, P], f32)
            make_identity(nc, ident)
            ident_bf = cst.tile([P, P], bf16)
            make_identity(nc, ident_bf)
            eps_t = cst.tile([P, 1], f32)
            nc.vector.memset(eps_t, EPS)
            ones_bf = cst.tile([P, P], bf16)
            nc.vector.memset(ones_bf, 1.0)

            # persistent activations
            qkT = glob.tile([P, 6, N_TOK], bf16)     # [d, head(0-4=q,5=k), n]
            v_all = glob.tile([P, NB, P], bf16)      # [n%128, n//128, d]
            oT_all = glob.tile([P, GQ, N_TOK], bf16)  # [d, head, n]
            gd = gdram.tile([GQ, N_TOK], f32)        # sigmoid(gate) rows

            # ---------------- phase A: rms + qkv GEMM + norms + rope ------
            with tc.tile_pool(name="paw", bufs=1) as paw, \
                 tc.tile_pool(name="pa2", bufs=2) as pa2, \
                 tc.tile_pool(name="pa1", bufs=1) as pa1, \
                 tc.tile_pool(name="paG", bufs=1) as paG, \
                 tc.tile_pool(name="psA", bufs=6, space="PSUM") as psA, \
                 tc.tile_pool(name="psT", bufs=2, space="PSUM") as psT:
                g_sig = paG.tile([GQ, N_TOK], f32)
                defer_q = {}            # tok0 -> (qrb tile, nt)
                KQ = KO // 4            # 10 ko per weight quarter

                def tile_input_dmas(ti, tok0, nt):
                    xt_t = pa2.tile([P, KO, P], bf16, tag="xt", name="xt_t")
                    nc.sync.dma_start(
                        out=xt_t[:, :, :nt],
                        in_=xt[xt_offs[ti]:xt_offs[ti] + P * KO * nt]
                        .rearrange("(p ko j) -> p ko j", p=P, ko=KO))
                    xn_t = pa1.tile([P, HIDDEN], bf16, tag="xn", name="xn_t")
                    nc.scalar.dma_start(out=xn_t[:nt], in_=xn[tok0:tok0 + nt])
                    rp_t = pa2.tile([P, 8, 64], f32, tag="rp", name="rp_t")
                    nc.sync.dma_start(out=rp_t[:nt],
                                      in_=ropec[tok0:tok0 + nt])
                    return xt_t, xn_t, rp_t

                pre_dma = {}
                for g in range(3):
                    # quarter the group weight so the next group's quarters
                    # stream in under this group's matmuls; the very first
                    # tile's inputs go ahead of the 9MB of g0 quarters so the
                    # PE isn't starved at kernel start, and quarter 0 is
                    # double-buffered to hide the group transition
                    if g == 0:
                        pre_dma[0] = tile_input_dmas(0, *tiles[0][:2])
                    wq_sb = []
                    for q in range(4):
                        wt = paw.tile([P, KQ, FC], bf16, tag=f"wq{q}",
                                      bufs=(2 if q == 0 else 1), name="wt")
                        nc.sync.dma_start(
                            out=wt[:],
                            in_=wqkv[g, q * KQ:(q + 1) * KQ]
                            .rearrange("ko p f -> p ko f"))
                        wq_sb.append(wt)
                    for ti, (tok0, nt, gg) in enumerate(tiles):
                        if gg != g:
                            continue
                        if ti in pre_dma:
                            xt_t, xn_t, rp_t = pre_dma.pop(ti)
                        else:
                            xt_t, xn_t, rp_t = tile_input_dmas(ti, tok0, nt)
                        # pre-norm rms (from raw x): sum(x^2) via ScalarE
                        # Square+accum, then sqrt(acc/H + eps), reciprocal
                        ssq = pa2.tile([P, 1], f32, tag="ssq")
                        nc.scalar.activation(out=xn_t[:nt], in_=xn_t[:nt],
                                             func=AF.Square,
                                             accum_out=ssq[:nt])
                        srt = pa2.tile([P, 1], f32, tag="srt")
                        nc.scalar.activation(srt[:nt], ssq[:nt], AF.Sqrt,
                                             scale=1.0 / HIDDEN,
                                             bias=eps_t[:nt])
                        rinv = pa2.tile([P, 1], f32, tag="rinv")
                        nc.vector.reciprocal(rinv[:nt], srt[:nt])
                        # qkv GEMM: psum [tokens, features]
                        ps_a = psA.tile([P, 512], f32, tag="ps512")
                        ps_b = psA.tile([P, 512], f32, tag="ps512")
                        for ko in range(KO):
                            wt = wq_sb[ko // KQ]
                            kq = ko % KQ
                            nc.tensor.matmul(
                                ps_a[:nt, :],
                                lhsT=xt_t[:, ko, :nt],
                                rhs=wt[:, kq, 0:512],
                                start=(ko == 0), stop=(ko == KO - 1))
                            nc.tensor.matmul(
                                ps_b[:nt, 0:FC - 512],
                                lhsT=xt_t[:, ko, :nt],
                                rhs=wt[:, kq, 512:FC],
                                start=(ko == 0), stop=(ko == KO - 1))
                        # evacuate with rms scale
                        qf = pa1.tile([P, GQ, HEAD_DIM], f32, tag="qf")
                        kf = pa1.tile([P, HEAD_DIM], f32, tag="kf")
                        vf = pa1.tile([P, HEAD_DIM], bf16, tag="vf")
                        gf = pa1.tile([P, 8], f32, tag="gf")
                        nc.vector.tensor_scalar_mul(
                            qf[:nt, 0:4, :], ps_a[:nt, :], rinv[:nt])
                        nc.vector.tensor_scalar_mul(
                            qf[:nt, 4, :], ps_b[:nt, 0:128], rinv[:nt])
                        nc.vector.tensor_scalar_mul(
                            kf[:nt, :], ps_b[:nt, 128:256], rinv[:nt])
                        nc.vector.tensor_scalar_mul(
                            vf[:nt, :], ps_b[:nt, 256:384], rinv[:nt])
                        nc.vector.tensor_scalar_mul(
                            gf[:nt, 0:GQ], ps_b[:nt, 384:389], rinv[:nt])
                        # v: straight into [n%128, n//128, d] via sbuf dma
                        o0, b0 = tok0 % P, tok0 // P
                        k1 = min(nt, P - o0)
                        nc.scalar.dma_start(out=v_all[o0:o0 + k1, b0, :],
                                            in_=vf[0:k1, :])
                        if nt > k1:
                            nc.scalar.dma_start(
                                out=v_all[0:nt - k1, b0 + 1, :],
                                in_=vf[k1:nt, :])
                        # q/k rms over head_dim (Square+accum per head)
                        sq = pa2.tile([P, 8], f32, tag="sq")
                        junk = pa1.tile([P, HEAD_DIM], f32, tag="junk")
                        for h in range(GQ):
                            nc.scalar.activation(
                                out=junk[:nt], in_=qf[:nt, h, :],
                                func=AF.Square,
                                accum_out=sq[:nt, h:h + 1])
                        nc.scalar.activation(
                            out=junk[:nt], in_=kf[:nt], func=AF.Square,
                            accum_out=sq[:nt, GQ:GQ + 1])
                        sqs = pa2.tile([P, 8], f32, tag="sqs")
                        nc.scalar.activation(sqs[:nt, 0:6], sq[:nt, 0:6],
                                             AF.Sqrt, scale=1.0 / HEAD_DIM,
                                             bias=eps_t[:nt])
                        rq = pa2.tile([P, 8], f32, tag="rq")
                        nc.vector.reciprocal(rq[:nt, 0:6], sqs[:nt, 0:6])
                        # rope+norm for q (coeff tables already fold w+1)
                        q1 = qf[:nt, :, 0:64]
                        q2 = qf[:nt, :, 64:128]
                        t1 = pa1.tile([P, GQ, 64], f32, tag="t1")
                        t2 = pa1.tile([P, GQ, 64], f32, tag="t2")
                        qr = pa2.tile([P, GQ, HEAD_DIM], f32, tag="qr")
                        # q rope output for tokens >= N2 is kept in SBUF and
                        # transposed during B(c0), filling PE exp-wait gaps
                        if tok0 >= N2:
                            qrb = paG.tile([P, GQ, HEAD_DIM], bf16,
                                           tag=f"dq{tok0 // P}", name="qrb")
                            defer_q[tok0] = (qrb, nt)
                        else:
                            qrb = pa2.tile([P, GQ, HEAD_DIM], bf16,
                                           tag="qrb", name="qrb")

                        def bc(i):
                            return rp_t[:nt, i:i + 1, :].to_broadcast(
                                (nt, GQ, 64))

                        nc.vector.tensor_tensor(t1[:nt], q1, bc(0), OP.mult)
                        nc.vector.tensor_tensor(t2[:nt], q2, bc(1), OP.mult)
                        nc.vector.tensor_tensor(qr[:nt, :, 0:64], t1[:nt],
                                                t2[:nt], OP.subtract)
                        nc.vector.tensor_tensor(t1[:nt], q1, bc(2), OP.mult)
                        nc.vector.tensor_tensor(t2[:nt], q2, bc(3), OP.mult)
                        nc.vector.tensor_tensor(qr[:nt, :, 64:128], t1[:nt],
                                                t2[:nt], OP.add)
                        nc.vector.tensor_tensor(
                            qrb[:nt], qr[:nt],
                            rq[:nt, 0:GQ, None].to_broadcast(
                                (nt, GQ, HEAD_DIM)), OP.mult)
                        # rope+norm for k
                        k1f = kf[:nt, 0:64]
                        k2f = kf[:nt, 64:128]
                        kr = pa2.tile([P, HEAD_DIM], f32, tag="kr")
                        krb = pa2.tile([P, HEAD_DIM], bf16, tag="krb")
                        t1k = pa1.tile([P, 64], f32, tag="t1k")
                        t2k = pa1.tile([P, 64], f32, tag="t2k")
                        nc.vector.tensor_tensor(t1k[:nt], k1f,
                                                rp_t[:nt, 4, :], OP.mult)
                        nc.vector.tensor_tensor(t2k[:nt], k2f,
                                                rp_t[:nt, 5, :], OP.mult)
                        nc.vector.tensor_tensor(kr[:nt, 0:64], t1k[:nt],
                                                t2k[:nt], OP.subtract)
                        nc.vector.tensor_tensor(t1k[:nt], k1f,
                                                rp_t[:nt, 6, :], OP.mult)
                        nc.vector.tensor_tensor(t2k[:nt], k2f,
                                                rp_t[:nt, 7, :], OP.mult)
                        nc.vector.tensor_tensor(kr[:nt, 64:128], t1k[:nt],
                                                t2k[:nt], OP.add)
                        nc.vector.tensor_scalar_mul(krb[:nt], kr[:nt],
                                                    rq[:nt, GQ:GQ + 1])
                        # transposes into [d, n] globals (bf16 streams 4x
                        # faster through the PE than f32)
                        if tok0 < N2:
                            for h in range(GQ):
                                tp = psT.tile([P, P], bf16, tag="tp")
                                nc.tensor.transpose(tp[:, :nt],
                                                    qrb[:nt, h, :],
                                                    ident_bf[:nt, :nt])
                                nc.vector.tensor_copy(
                                    out=qkT[:, h, tok0:tok0 + nt],
                                    in_=tp[:, :nt])
                        tp = psT.tile([P, P], bf16, tag="tp")
                        nc.tensor.transpose(tp[:, :nt], krb[:nt],
                                            ident_bf[:nt, :nt])
                        nc.vector.tensor_copy(out=qkT[:, GQ, tok0:tok0 + nt],
                                              in_=tp[:, :nt])
                        tpg = psT.tile([P, P], f32, tag="tp")
                        nc.tensor.transpose(tpg[0:GQ, :nt], gf[:nt, 0:GQ],
                                            ident[:nt, :nt])
                        nc.vector.tensor_copy(out=g_sig[0:GQ, tok0:tok0 + nt],
                                              in_=tpg[0:GQ, :nt])
                # gate rows -> ln(sigmoid) -> DRAM (per-head broadcast reads
                # them back in phase B; engines only address partition 0 up)
                nc.scalar.activation(g_sig[0:GQ, :], g_sig[0:GQ, :],
                                     AF.Sigmoid)
                nc.scalar.activation(g_sig[0:GQ, :], g_sig[0:GQ, :], AF.Ln)
                nc.sync.dma_start(out=gd[:], in_=g_sig[0:GQ, :])

            # ---------------- phase B+C: attention fused with projection ---
            with tc.tile_pool(name="pb", bufs=1) as pb, \
                 tc.tile_pool(name="pcw", bufs=6) as pcw, \
                 tc.tile_pool(name="pco", bufs=2) as pco, \
                 tc.tile_pool(name="psS", bufs=2, space="PSUM") as psS, \
                 tc.tile_pool(name="psO", bufs=1, space="PSUM") as psO, \
                 tc.tile_pool(name="psC", bufs=2, space="PSUM") as psC:
                # per-head ln(sigmoid) gate rows broadcast to all partitions
                lsig_b = []
                for h in range(GQ):
                    sb = pb.tile([P, N_TOK], f32, tag=f"sb{h}", name=f"sb{h}")
                    nc.sync.dma_start(
                        out=sb[:], in_=gd[h:h + 1, :].to_broadcast((P, N_TOK)))
                    lsig_b.append(sb)

                # ---- proj work units (streamed weights, batched output) ---
                wp_tiles = {}

                def c_prefetch(k, units, eng=None):
                    if k < len(units) and k not in wp_tiles:
                        ci, ht = units[k]
                        g = csegs[ci][1]
                        wt = pcw.tile([P, GQ, P], bf16, tag="wp", name="wt")
                        (eng or nc.sync).dma_start(
                            out=wt[:],
                            in_=wproj[g, ht].rearrange("f d h -> d f h"))
                        wp_tiles[k] = wt

                ob_cur = [None, 0]      # tile, base token

                def emit_c_unit(k, units, eng=None):
                    ci, ht = units[k]
                    seg, g, base, rngs = csegs[ci]
                    for kk in range(k + 1, k + 5):
                        c_prefetch(kk, units, eng)
                    wt = wp_tiles.pop(k)
                    if ht % 4 == 0:
                        ob_cur[0] = pco.tile([P, 4, ob_max], bf16, tag="ob",
                                             name="ob")
                        ob_cur[1] = base
                    ob = ob_cur[0]
                    for (a, b) in rngs:
                        po = psC.tile([P, 512], f32, tag="po", name="po")
                        for f in range(GQ):
                            nc.tensor.matmul(
                                po[:, :b - a],
                                lhsT=wt[:, f, :],
                                rhs=oT_all[:, f, a:b],
                                start=(f == 0), stop=(f == GQ - 1))
                        dst = ob[:, ht % 4, a - base:b - base]
                        if ht % 2 == 0:
                            nc.vector.tensor_copy(out=dst, in_=po[:, :b - a])
                        else:
                            nc.scalar.copy(out=dst, in_=po[:, :b - a])
                    if ht % 4 == 3:
                        ctot = rngs[-1][1] - base
                        nc.sync.dma_start(
                            out=outT[(ht - 3) * P:(ht + 1) * P,
                                     base:base + ctot]
                            .rearrange("(o p) n -> p o n", p=P),
                            in_=ob[:, :, :ctot])

                # ---- attention for one (c, h) --------------------------
                def emit_b(c, h, inject):
                    nsl = slice(c * N2, (c + 1) * N2)
                    o_ps = psO.tile([P, N2], f32, tag="o", name="o_ps")
                    acc = pb.tile([P, N2], bf16, tag="acc", bufs=2,
                                  name="acc")
                    for m in range(NB):
                        s_ps = psS.tile([P, N2], f32, tag="s", name="s_ps")
                        for u in range(N2 // 512):
                            nc.tensor.matmul(
                                s_ps[:, u * 512:(u + 1) * 512],
                                lhsT=qkT[:, GQ, m * P:(m + 1) * P],
                                rhs=qkT[:, h, c * N2 + u * 512:
                                        c * N2 + (u + 1) * 512],
                                start=True, stop=True)
                        pT = pb.tile([P, N2], bf16, tag="pT", bufs=3,
                                     name="pT")
                        nc.scalar.activation(pT[:], s_ps[:], AF.Exp,
                                             scale=SCALE)
                        for u in range(N2 // 512):
                            usl = slice(u * 512, (u + 1) * 512)
                            nc.tensor.matmul(
                                o_ps[:, usl], lhsT=v_all[:, m, :],
                                rhs=pT[:, usl],
                                start=(m == 0), stop=(m == NB - 1))
                        if m == 0:
                            nc.vector.tensor_copy(out=acc[:], in_=pT[:])
                        else:
                            nc.vector.tensor_tensor(acc[:], acc[:],
                                                    pT[:], OP.add)
                        if inject is not None:
                            inject()
                    # denominator: ones-matmul column-sums acc; the 128-wide
                    # ones stationary broadcasts the row to every partition.
                    # sig/den = exp(-(ln den - ln sig)) avoids the slow DVE
                    # reciprocal; exp+ln live in one scalar table set.
                    den_ps = psS.tile([P, N2], f32, tag="s", name="den_ps")
                    for u in range(N2 // 512):
                        usl = slice(u * 512, (u + 1) * 512)
                        nc.tensor.matmul(den_ps[:, usl], lhsT=ones_bf[:],
                                         rhs=acc[:, usl],
                                         start=True, stop=True)
                    lt = pb.tile([P, N2], f32, tag="lt", bufs=2, name="lt")
                    nc.scalar.activation(lt[:], den_ps[:], AF.Ln)
                    nc.vector.tensor_tensor(lt[:], lt[:],
                                            lsig_b[h][:, nsl], OP.subtract)
                    dg = pb.tile([P, N2], bf16, tag="dg", bufs=2, name="dg")
                    nc.scalar.activation(dg[:], lt[:], AF.Exp, scale=-1.0)
                    nc.vector.tensor_tensor(oT_all[:, h, nsl], o_ps[:],
                                            dg[:], OP.mult)

                # ---- fused schedule ------------------------------------
                # deferred q transposes (tokens >= N2) fill B(c0) PE gaps
                dq_work = [(tok0, h) for tok0 in sorted(defer_q)
                           for h in range(GQ)]
                dctr = [0]

                def inject_defer():
                    if dctr[0] < len(dq_work):
                        tok0, h = dq_work[dctr[0]]
                        dctr[0] += 1
                        qrb, nt = defer_q[tok0]
                        tp = psC.tile([P, P], bf16, tag="po", name="tp")
                        nc.tensor.transpose(tp[:, :nt], qrb[:nt, h, :],
                                            ident_bf[:nt, :nt])
                        nc.vector.tensor_copy(
                            out=qkT[:, h, tok0:tok0 + nt], in_=tp[:, :nt])

                for h in range(GQ):
                    emit_b(0, h, inject_defer)
                while dctr[0] < len(dq_work):
                    inject_defer()
                for k in range(4):
                    c_prefetch(k, cunits_s0)
                ctr = [0]

                def inject_s0():
                    if ctr[0] < len(cunits_s0):
                        emit_c_unit(ctr[0], cunits_s0)
                        ctr[0] += 1

                for h in range(GQ):
                    emit_b(1, h, inject_s0)
                while ctr[0] < len(cunits_s0):
                    inject_s0()
                for k in range(4):
                    c_prefetch(k, cunits_s1, nc.scalar)
                for k in range(len(cunits_s1)):
                    emit_c_unit(k, cunits_s1, nc.scalar)

    return nc, tiles, xt_offs, xt_total


# ---------------------------------------------------------------------------
# host wrapper
# ---------------------------------------------------------------------------

def prepare(hidden_states, rope, pre_norm_w, qkv_w, q_norm_w, k_norm_w,
            proj_w, modality_ids):
    """Host-side layout prep. Returns (counts, perm, in_maps_fn) where
    in_maps_fn(tiles, xt_offs, xt_total) builds the per-core input maps."""
    import ml_dtypes

    bf16 = ml_dtypes.bfloat16
    x = np.asarray(hidden_states, np.float32)
    rope = np.asarray(rope, np.float32)
    pre_w = np.asarray(pre_norm_w, np.float32).reshape(NUM_MOD, HIDDEN)
    qkv_w = np.asarray(qkv_w, np.float32).reshape(NUM_MOD, QKV_OUT, HIDDEN)
    qn_w = np.asarray(q_norm_w, np.float32).reshape(NUM_MOD, HEAD_DIM)
    kn_w = np.asarray(k_norm_w, np.float32).reshape(NUM_MOD, HEAD_DIM)
    proj_w = np.asarray(proj_w, np.float32).reshape(NUM_MOD, HIDDEN, Q_SIZE)
    mids = np.asarray(modality_ids).astype(np.int64)

    perm = np.argsort(mids, kind="stable")
    counts = tuple(int((mids == g).sum()) for g in range(NUM_MOD))
    x_p = x[perm]
    rope_p = rope[perm]
    mids_p = mids[perm]

    # ---- rope coefficient tables (fold q/k-norm w+1) ----
    sin = rope_p[:, :64]
    cos = rope_p[:, 64:]
    wq = qn_w[mids_p] + 1.0                             # [N, 128]
    wk = kn_w[mids_p] + 1.0
    ropec = np.empty((N_TOK, 8, 64), np.float32)
    ropec[:, 0] = cos * wq[:, :64]
    ropec[:, 1] = sin * wq[:, 64:]
    ropec[:, 2] = sin * wq[:, :64]
    ropec[:, 3] = cos * wq[:, 64:]
    ropec[:, 4] = cos * wk[:, :64]
    ropec[:, 5] = sin * wk[:, 64:]
    ropec[:, 6] = sin * wk[:, :64]
    ropec[:, 7] = cos * wk[:, 64:]

    # ---- per-core weight slices ----
    wqkv_cores = []
    wproj_cores = []
    for c in range(NCORES):
        rows = np.concatenate([
            np.arange(c * QC, (c + 1) * QC),
            np.arange(Q_SIZE + c * HEAD_DIM, Q_SIZE + (c + 1) * HEAD_DIM),
            np.arange(Q_SIZE + KV_SIZE + c * HEAD_DIM,
                      Q_SIZE + KV_SIZE + (c + 1) * HEAD_DIM),
            np.arange(Q_SIZE + 2 * KV_SIZE + c * GQ,
                      Q_SIZE + 2 * KV_SIZE + (c + 1) * GQ),
        ])
        wc = qkv_w[:, rows, :] * (pre_w[:, None, :] + 1.0)  # [3, 901, 5120]
        wt = wc.transpose(0, 2, 1).reshape(NUM_MOD, KO, P, FC)
        wqkv_cores.append(np.ascontiguousarray(wt).astype(bf16))
        # proj slice: [3, 40, 5, 128(d), 128(hcol)]
        pc = proj_w[:, :, c * QC:(c + 1) * QC]              # [3, 5120, 640]
        pt = pc.reshape(NUM_MOD, HT, P, GQ, HEAD_DIM).transpose(0, 1, 3, 4, 2)
        wproj_cores.append(np.ascontiguousarray(pt).astype(bf16))

    x_bf = x_p.astype(bf16)

    def in_maps_fn(tiles, xt_offs, xt_total):
        xt_flat = np.empty(xt_total, bf16)
        for (tok0, nt, g), off in zip(tiles, xt_offs):
            blk = x_bf[tok0:tok0 + nt]                    # [nt, 5120]
            t = blk.reshape(nt, KO, P).transpose(2, 1, 0)  # [p, ko, nt]
            xt_flat[off:off + P * KO * nt] = \
                np.ascontiguousarray(t).reshape(-1)
        return [{
            "xt": xt_flat,
            "xn": x_bf,
            "ropec": ropec,
            "wqkv": wqkv_cores[c],
            "wproj": wproj_cores[c],
        } for c in range(NCORES)]

    return counts, perm, in_maps_fn


def kernel(hidden_states, rope, pre_norm_w, qkv_w, q_norm_w, k_norm_w,
           proj_w, modality_ids):
    global LAST_EXEC_NS

    counts, perm, in_maps_fn = prepare(
        hidden_states, rope, pre_norm_w, qkv_w, q_norm_w, k_norm_w,
        proj_w, modality_ids)

    if counts not in _BUILD_CACHE:
        _install_profile_hook()
        _install_legalizer()
        _BUILD_CACHE[counts] = _build(counts)
    nc, tiles, xt_offs, xt_total = _BUILD_CACHE[counts]

    in_maps = in_maps_fn(tiles, xt_offs, xt_total)

    from concourse.bass_utils import run_bass_kernel_spmd

    trace = os.environ.get("BASSMOE_TRACE", "") == "1"
    res = run_bass_kernel_spmd(nc, in_maps, core_ids=list(range(NCORES)),
                               trace=trace)
    LAST_EXEC_NS = res.exec_time_ns

    acc = np.zeros((HIDDEN, N_TOK), np.float64)
    for c in range(NCORES):
        acc += np.asarray(res.results[c]["outT"], np.float64)
    out_p = acc.T.astype(np.float32)                    # [N, HIDDEN] permuted
    out = np.empty_like(out_p)
    out[perm] = out_p
    return out


# revision 20
# speedup vs baseline: 1.3729x; 1.0456x over previous
"""DaVinci attention (multi-modal MoE-routed attention block) on 8 Trainium2
NeuronCores.

Sharding: tensor-parallel over heads.  Each of the 8 cores owns one KV head
and its 5 GQA query heads: qkv-weight columns (640 q + 128 k + 128 v + 5 gate
per core) and proj-weight rows (640 per core) are sliced per core; the final
projection output is a partial sum reduced on the host.

Host-side prep (layout only — all FLOPs stay on device):
  * tokens are permuted so same-modality tokens are contiguous; each expert's
    GEMM then runs on its own token range (no 3x masked-dispatch waste)
  * pre-norm weight (w+1) is folded into the qkv weight columns; the
    per-token rms scale is applied on-device after the GEMM
  * q/k-norm weights (w+1) are folded into host-precomputed rope coefficient
    tables A=cos*(w1+1), B=sin*(w2+1), D=sin*(w1+1), E=cos*(w2+1)
  * weights are pre-transposed/tiled for contraction-major DMA

Schedule: phase A (rms + qkv GEMM + norms + rope) fills qkT/v_all; phase B
(attention) and phase C (output projection) are fused — C's GEMMs for the
first half of the sequence are interleaved under B's second half so the PE
array stays busy while the scalar engine runs softmax exps.  The softmax
denominator row is produced+broadcast in one gpsimd partition_all_reduce and
inverted with the fast DVE reciprocal; sigmoid gate rows are broadcast once
per head.
"""

import os
import sys
import types

import numpy as np

HIDDEN = 5120
HEAD_DIM = 128
HQ = 40
HKV = 8
NUM_MOD = 3
Q_SIZE = HQ * HEAD_DIM          # 5120
KV_SIZE = HKV * HEAD_DIM        # 1024
GATE = HQ
QKV_OUT = Q_SIZE + 2 * KV_SIZE + GATE  # 7208
EPS = 1e-6
N_TOK = 2048
P = 128
NCORES = 8
GQ = HQ // HKV                  # 5 q heads per core
QC = GQ * HEAD_DIM              # 640 q cols per core
FC = QC + 2 * HEAD_DIM + GQ     # 901 qkv out features per core
KO = HIDDEN // P                # 40 contraction chunks
NB = N_TOK // P                 # 16 token blocks of 128 (attention tiling)
N2 = 1024                       # attention free-dim chunk
HT = HIDDEN // P                # 40 proj output row-blocks
SCALE = 1.0 / float(np.sqrt(HEAD_DIM))

LAST_EXEC_NS = None             # filled when BASSMOE_TRACE=1


# ---------------------------------------------------------------------------
# axon NTFF profiling hook (needed only when tracing) + BIR sync legalizer
# ---------------------------------------------------------------------------

def _install_profile_hook():
    if "antenv.axon_hooks" in sys.modules:
        return
    mod = types.ModuleType("antenv.axon_hooks")
    _h = [None]
    mod.set_axon_ntff_profile_hook = lambda h: _h.__setitem__(0, h)
    mod.get_axon_ntff_profile_hook = lambda: _h[0]
    import antenv

    antenv.axon_hooks = mod
    sys.modules["antenv.axon_hooks"] = mod
    try:
        from trn_agent_boot.trn_boot import _ntff_profile_via_ctypes

        mod.set_axon_ntff_profile_hook(
            _ntff_profile_via_ctypes("/opt/axon/libaxon_pjrt.so")
        )
    except Exception:
        pass


def _legalize_sync(bir_json):
    """This walrus build accepts a single sync wait/update per instruction.
    Move extra waits onto preceding same-engine NoOps (the engine stalls
    before dispatch either way) and extra updates onto trailing NoOps."""
    import json

    data = json.loads(bir_json)
    for fn in data["functions"]:
        for blk in fn["blocks"]:
            out = []
            for ins in blk["instructions"]:
                si = ins.get("sync_info")
                waits = si.get("on_wait", []) if si else []
                upds = si.get("on_update", []) if si else []
                if len(waits) > 1:
                    for i, w in enumerate(waits[:-1]):
                        out.append({
                            "debug": ins.get("debug", 0),
                            "engine": ins["engine"],
                            "ins": [], "is_reset_sema": False,
                            "name": f"{ins['name']}-lw{i}",
                            "opcode": "NoOp", "outs": [],
                            "sync_info": {"on_update": [], "on_wait": [w]},
                        })
                    si["on_wait"] = [waits[-1]]
                out.append(ins)
                if len(upds) > 1:
                    if ins["opcode"] in ("DMACopy", "DMATranspose"):
                        raise AssertionError(
                            f"DMA instruction {ins['name']} has multiple updates")
                    for i, u in enumerate(upds[1:]):
                        out.append({
                            "debug": ins.get("debug", 0),
                            "engine": ins["engine"],
                            "ins": [], "is_reset_sema": False,
                            "name": f"{ins['name']}-lu{i}",
                            "opcode": "NoOp", "outs": [],
                            "sync_info": {"on_update": [u], "on_wait": []},
                        })
                    si["on_update"] = [upds[0]]
            blk["instructions"] = out
    return json.dumps(data).encode()


def _install_legalizer():
    from concourse import bass2jax, bass_utils

    if getattr(bass2jax, "_sync_legalize_installed", False):
        return
    orig = bass_utils.compile_bir_kernel

    def wrapped(bir_json, tmpdir, neff_name="file.neff"):
        return orig(_legalize_sync(bir_json), tmpdir, neff_name)

    bass2jax.compile_bir_kernel = wrapped
    bass_utils.compile_bir_kernel = wrapped
    bass2jax._sync_legalize_installed = True


# ---------------------------------------------------------------------------
# device program
# ---------------------------------------------------------------------------

_BUILD_CACHE = {}


def _build(counts):
    import concourse.bass as bass
    import concourse.tile as tile
    from concourse import mybir, bass_isa
    from concourse.masks import make_identity

    f32 = mybir.dt.float32
    bf16 = mybir.dt.bfloat16
    AF = mybir.ActivationFunctionType
    OP = mybir.AluOpType

    n0, n1, n2 = counts
    starts = [0, n0, n0 + n1, 2048]
    # group-chunked qkv tiles (tok0, nt, g)
    tiles = []
    for g in range(3):
        t0, t1 = starts[g], starts[g + 1]
        for a in range(t0, t1, P):
            tiles.append((a, min(P, t1 - a), g))
    # packed-xt flat offsets per tile
    xt_offs = []
    off = 0
    for (a, nt, g) in tiles:
        xt_offs.append(off)
        off += P * KO * nt
    xt_total = off

    # proj work: per B-segment (N2 tokens), per group, the contiguous token
    # ranges (split <=512 for PSUM); each (seg, g) then sweeps 40 ht blocks
    csegs = []                       # (seg, g, base, [(a, b), ...])
    for seg in range(N_TOK // N2):
        lo, hi = seg * N2, (seg + 1) * N2
        for g in range(3):
            a0, b0 = max(lo, starts[g]), min(hi, starts[g + 1])
            if a0 < b0:
                rngs = [(a, min(a + 512, b0)) for a in range(a0, b0, 512)]
                csegs.append((seg, g, a0, rngs))
    cunits = [(ci, ht) for ci, cs in enumerate(csegs) for ht in range(HT)]
    cunits_s0 = [(ci, ht) for (ci, ht) in cunits if csegs[ci][0] == 0]
    cunits_s1 = [(ci, ht) for (ci, ht) in cunits if csegs[ci][0] == 1]
    ob_max = max(cs[3][-1][1] - cs[2] for cs in csegs)

    nc = bass.Bass()
    xt = nc.dram_tensor("xt", (xt_total,), bf16, kind="ExternalInput")
    xn = nc.dram_tensor("xn", (N_TOK, HIDDEN), bf16, kind="ExternalInput")
    ropec = nc.dram_tensor("ropec", (N_TOK, 8, 64), f32, kind="ExternalInput")
    wqkv = nc.dram_tensor("wqkv", (NUM_MOD, KO, P, FC), bf16, kind="ExternalInput")
    wproj = nc.dram_tensor("wproj", (NUM_MOD, HT, GQ, P, P), bf16,
                           kind="ExternalInput")
    outT = nc.dram_tensor("outT", (HIDDEN, N_TOK), bf16, kind="ExternalOutput")

    with tile.TileContext(nc) as tc:
        with tc.tile_pool(name="cst", bufs=1) as cst, \
             tc.tile_pool(name="gdram", bufs=1, space="DRAM") as gdram, \
             tc.tile_pool(name="glob", bufs=1) as glob:
            ident = cst.tile([P, P], f32)
            make_identity(nc, ident)
            ident_bf = cst.tile([P, P], bf16)
            make_identity(nc, ident_bf)
            eps_t = cst.tile([P, 1], f32)
            nc.vector.memset(eps_t, EPS)
            ones_bf = cst.tile([P, P], bf16)
            nc.vector.memset(ones_bf, 1.0)

            # persistent activations
            qkT = glob.tile([P, 6, N_TOK], bf16)     # [d, head(0-4=q,5=k), n]
            v_all = glob.tile([P, NB, P], bf16)      # [n%128, n//128, d]
            gd = gdram.tile([GQ, N_TOK], f32)        # ln(sigmoid(gate)) rows

            # ---------------- phase A: rms + qkv GEMM + norms + rope ------
            with tc.tile_pool(name="paw", bufs=1) as paw, \
                 tc.tile_pool(name="pa2", bufs=2) as pa2, \
                 tc.tile_pool(name="pa1", bufs=1) as pa1, \
                 tc.tile_pool(name="paG", bufs=1) as paG, \
                 tc.tile_pool(name="psA", bufs=6, space="PSUM") as psA, \
                 tc.tile_pool(name="psT", bufs=2, space="PSUM") as psT:
                g_sig = paG.tile([GQ, N_TOK], f32)
                defer_q = {}            # tok0 -> (qrb tile, nt)
                KQ = KO // 4            # 10 ko per weight quarter

                def tile_input_dmas(ti, tok0, nt):
                    xt_t = pa2.tile([P, KO, P], bf16, tag="xt", bufs=3,
                                    name="xt_t")
                    nc.sync.dma_start(
                        out=xt_t[:, :, :nt],
                        in_=xt[xt_offs[ti]:xt_offs[ti] + P * KO * nt]
                        .rearrange("(p ko j) -> p ko j", p=P, ko=KO))
                    xn_t = pa1.tile([P, HIDDEN], bf16, tag="xn", bufs=2,
                                    name="xn_t")
                    nc.scalar.dma_start(out=xn_t[:nt], in_=xn[tok0:tok0 + nt])
                    rp_t = pa2.tile([P, 8, 64], f32, tag="rp", name="rp_t")
                    nc.sync.dma_start(out=rp_t[:nt],
                                      in_=ropec[tok0:tok0 + nt])
                    return xt_t, xn_t, rp_t

                pre_dma = {}
                for g in range(3):
                    # pre-issue the group's first two tiles' inputs so they
                    # don't queue behind the 9MB of weight quarters on the
                    # sync DMA ring; quarter 0 is double-buffered so the next
                    # group's first matmuls aren't starved either
                    gtis = [ti for ti, t in enumerate(tiles) if t[2] == g]
                    for ti in gtis[:2]:
                        pre_dma[ti] = tile_input_dmas(ti, *tiles[ti][:2])
                    wq_sb = []
                    for q in range(4):
                        wt = paw.tile([P, KQ, FC], bf16, tag=f"wq{q}",
                                      bufs=(2 if q == 0 else 1), name="wt")
                        if g == 0 and q == 0:
                            # halve the very first load: the first matmul
                            # only needs ko 0-4
                            h2 = KQ // 2
                            nc.sync.dma_start(
                                out=wt[:, 0:h2],
                                in_=wqkv[g, 0:h2]
                                .rearrange("ko p f -> p ko f"))
                            nc.sync.dma_start(
                                out=wt[:, h2:KQ],
                                in_=wqkv[g, h2:KQ]
                                .rearrange("ko p f -> p ko f"))
                        else:
                            nc.sync.dma_start(
                                out=wt[:],
                                in_=wqkv[g, q * KQ:(q + 1) * KQ]
                                .rearrange("ko p f -> p ko f"))
                        wq_sb.append(wt)
                    for ti, (tok0, nt, gg) in enumerate(tiles):
                        if gg != g:
                            continue
                        if ti in pre_dma:
                            xt_t, xn_t, rp_t = pre_dma.pop(ti)
                        else:
                            xt_t, xn_t, rp_t = tile_input_dmas(ti, tok0, nt)
                        # pre-norm rms (from raw x): sum(x^2) via ScalarE
                        # Square+accum, then sqrt(acc/H + eps), reciprocal
                        ssq = pa2.tile([P, 1], f32, tag="ssq")
                        nc.scalar.activation(out=xn_t[:nt], in_=xn_t[:nt],
                                             func=AF.Square,
                                             accum_out=ssq[:nt])
                        srt = pa2.tile([P, 1], f32, tag="srt")
                        nc.scalar.activation(srt[:nt], ssq[:nt], AF.Sqrt,
                                             scale=1.0 / HIDDEN,
                                             bias=eps_t[:nt])
                        rinv = pa2.tile([P, 1], f32, tag="rinv")
                        nc.vector.reciprocal(rinv[:nt], srt[:nt])
                        # qkv GEMM: psum [tokens, features]
                        ps_a = psA.tile([P, 512], f32, tag="ps512")
                        ps_b = psA.tile([P, 512], f32, tag="ps512")
                        for ko in range(KO):
                            wt = wq_sb[ko // KQ]
                            kq = ko % KQ
                            nc.tensor.matmul(
                                ps_a[:nt, :],
                                lhsT=xt_t[:, ko, :nt],
                                rhs=wt[:, kq, 0:512],
                                start=(ko == 0), stop=(ko == KO - 1))
                            nc.tensor.matmul(
                                ps_b[:nt, 0:FC - 512],
                                lhsT=xt_t[:, ko, :nt],
                                rhs=wt[:, kq, 512:FC],
                                start=(ko == 0), stop=(ko == KO - 1))
                        # evacuate with rms scale
                        qf = pa1.tile([P, GQ, HEAD_DIM], f32, tag="qf")
                        kf = pa1.tile([P, HEAD_DIM], f32, tag="kf")
                        vf = pa1.tile([P, HEAD_DIM], bf16, tag="vf")
                        gf = pa1.tile([P, 8], f32, tag="gf")
                        nc.vector.tensor_scalar_mul(
                            qf[:nt, 0:4, :], ps_a[:nt, :], rinv[:nt])
                        nc.vector.tensor_scalar_mul(
                            qf[:nt, 4, :], ps_b[:nt, 0:128], rinv[:nt])
                        nc.vector.tensor_scalar_mul(
                            kf[:nt, :], ps_b[:nt, 128:256], rinv[:nt])
                        nc.vector.tensor_scalar_mul(
                            vf[:nt, :], ps_b[:nt, 256:384], rinv[:nt])
                        nc.vector.tensor_scalar_mul(
                            gf[:nt, 0:GQ], ps_b[:nt, 384:389], rinv[:nt])
                        # v: straight into [n%128, n//128, d] via sbuf dma
                        o0, b0 = tok0 % P, tok0 // P
                        k1 = min(nt, P - o0)
                        nc.scalar.dma_start(out=v_all[o0:o0 + k1, b0, :],
                                            in_=vf[0:k1, :])
                        if nt > k1:
                            nc.scalar.dma_start(
                                out=v_all[0:nt - k1, b0 + 1, :],
                                in_=vf[k1:nt, :])
                        # q/k rms over head_dim (Square+accum per head)
                        sq = pa2.tile([P, 8], f32, tag="sq")
                        junk = pa1.tile([P, HEAD_DIM], f32, tag="junk")
                        for h in range(GQ):
                            nc.scalar.activation(
                                out=junk[:nt], in_=qf[:nt, h, :],
                                func=AF.Square,
                                accum_out=sq[:nt, h:h + 1])
                        nc.scalar.activation(
                            out=junk[:nt], in_=kf[:nt], func=AF.Square,
                            accum_out=sq[:nt, GQ:GQ + 1])
                        sqs = pa2.tile([P, 8], f32, tag="sqs")
                        nc.scalar.activation(sqs[:nt, 0:6], sq[:nt, 0:6],
                                             AF.Sqrt, scale=1.0 / HEAD_DIM,
                                             bias=eps_t[:nt])
                        rq = pa2.tile([P, 8], f32, tag="rq")
                        nc.vector.reciprocal(rq[:nt, 0:6], sqs[:nt, 0:6])
                        # rope+norm for q (coeff tables already fold w+1)
                        q1 = qf[:nt, :, 0:64]
                        q2 = qf[:nt, :, 64:128]
                        t1 = pa1.tile([P, GQ, 64], f32, tag="t1")
                        t2 = pa1.tile([P, GQ, 64], f32, tag="t2")
                        qr = pa2.tile([P, GQ, HEAD_DIM], f32, tag="qr")
                        # q rope output for tokens >= N2 is kept in SBUF and
                        # transposed during B(c0), filling PE exp-wait gaps
                        if tok0 >= N2:
                            qrb = paG.tile([P, GQ, HEAD_DIM], bf16,
                                           tag=f"dq{tok0 // P}", name="qrb")
                            defer_q[tok0] = (qrb, nt)
                        else:
                            qrb = pa2.tile([P, GQ, HEAD_DIM], bf16,
                                           tag="qrb", name="qrb")

                        def bc(i):
                            return rp_t[:nt, i:i + 1, :].to_broadcast(
                                (nt, GQ, 64))

                        nc.vector.tensor_tensor(t1[:nt], q1, bc(0), OP.mult)
                        nc.vector.tensor_tensor(t2[:nt], q2, bc(1), OP.mult)
                        nc.vector.tensor_tensor(qr[:nt, :, 0:64], t1[:nt],
                                                t2[:nt], OP.subtract)
                        nc.vector.tensor_tensor(t1[:nt], q1, bc(2), OP.mult)
                        nc.vector.tensor_tensor(t2[:nt], q2, bc(3), OP.mult)
                        nc.vector.tensor_tensor(qr[:nt, :, 64:128], t1[:nt],
                                                t2[:nt], OP.add)
                        nc.vector.tensor_tensor(
                            qrb[:nt], qr[:nt],
                            rq[:nt, 0:GQ, None].to_broadcast(
                                (nt, GQ, HEAD_DIM)), OP.mult)
                        # rope+norm for k
                        k1f = kf[:nt, 0:64]
                        k2f = kf[:nt, 64:128]
                        kr = pa2.tile([P, HEAD_DIM], f32, tag="kr")
                        krb = pa2.tile([P, HEAD_DIM], bf16, tag="krb")
                        t1k = pa1.tile([P, 64], f32, tag="t1k")
                        t2k = pa1.tile([P, 64], f32, tag="t2k")
                        nc.vector.tensor_tensor(t1k[:nt], k1f,
                                                rp_t[:nt, 4, :], OP.mult)
                        nc.vector.tensor_tensor(t2k[:nt], k2f,
                                                rp_t[:nt, 5, :], OP.mult)
                        nc.vector.tensor_tensor(kr[:nt, 0:64], t1k[:nt],
                                                t2k[:nt], OP.subtract)
                        nc.vector.tensor_tensor(t1k[:nt], k1f,
                                                rp_t[:nt, 6, :], OP.mult)
                        nc.vector.tensor_tensor(t2k[:nt], k2f,
                                                rp_t[:nt, 7, :], OP.mult)
                        nc.vector.tensor_tensor(kr[:nt, 64:128], t1k[:nt],
                                                t2k[:nt], OP.add)
                        nc.vector.tensor_scalar_mul(krb[:nt], kr[:nt],
                                                    rq[:nt, GQ:GQ + 1])
                        # transposes into [d, n] globals (bf16 streams 4x
                        # faster through the PE than f32)
                        if tok0 < N2:
                            for h in range(GQ):
                                tp = psT.tile([P, P], bf16, tag="tp")
                                nc.tensor.transpose(tp[:, :nt],
                                                    qrb[:nt, h, :],
                                                    ident_bf[:nt, :nt])
                                nc.vector.tensor_copy(
                                    out=qkT[:, h, tok0:tok0 + nt],
                                    in_=tp[:, :nt])
                        tp = psT.tile([P, P], bf16, tag="tp")
                        nc.tensor.transpose(tp[:, :nt], krb[:nt],
                                            ident_bf[:nt, :nt])
                        nc.vector.tensor_copy(out=qkT[:, GQ, tok0:tok0 + nt],
                                              in_=tp[:, :nt])
                        tpg = psT.tile([P, P], f32, tag="tp")
                        nc.tensor.transpose(tpg[0:GQ, :nt], gf[:nt, 0:GQ],
                                            ident[:nt, :nt])
                        nc.vector.tensor_copy(out=g_sig[0:GQ, tok0:tok0 + nt],
                                              in_=tpg[0:GQ, :nt])
                # gate rows -> ln(sigmoid) -> DRAM (per-head broadcast reads
                # them back in phase B; engines only address partition 0 up)
                nc.scalar.activation(g_sig[0:GQ, :], g_sig[0:GQ, :],
                                     AF.Sigmoid)
                nc.scalar.activation(g_sig[0:GQ, :], g_sig[0:GQ, :], AF.Ln)
                nc.sync.dma_start(out=gd[:], in_=g_sig[0:GQ, :])

            # ---------------- phase B+C: attention fused with projection ---
            with tc.tile_pool(name="pb", bufs=1) as pb, \
                 tc.tile_pool(name="pcw", bufs=6) as pcw, \
                 tc.tile_pool(name="pco", bufs=2) as pco, \
                 tc.tile_pool(name="psC", bufs=2, space="PSUM") as psC:
                oT_all = pb.tile([P, GQ, N_TOK], bf16)  # [d, head, n]
                # per-head ln(sigmoid) gate rows broadcast to all partitions
                lsig_b = []
                for h in range(GQ):
                    sb = pb.tile([P, N_TOK], f32, tag=f"sb{h}", name=f"sb{h}")
                    nc.sync.dma_start(
                        out=sb[:], in_=gd[h:h + 1, :].to_broadcast((P, N_TOK)))
                    lsig_b.append(sb)

                # ---- proj work units (streamed weights, batched output) ---
                wp_tiles = {}

                def c_prefetch(k, units, engf=None):
                    if k < len(units) and k not in wp_tiles:
                        ci, ht = units[k]
                        g = csegs[ci][1]
                        wt = pcw.tile([P, GQ, P], bf16, tag="wp", name="wt")
                        eng = engf(k) if engf else nc.sync
                        eng.dma_start(
                            out=wt[:],
                            in_=wproj[g, ht].rearrange("f d h -> d f h"))
                        wp_tiles[k] = wt

                ob_cur = [None, 0]      # tile, base token
                po_ctr = [0]

                def emit_c_unit(k, units, engf=None, pools=None):
                    ci, ht = units[k]
                    seg, g, base, rngs = csegs[ci]
                    for kk in range(k + 1, k + 5):
                        c_prefetch(kk, units, engf)
                    wt = wp_tiles.pop(k)
                    if ht % 4 == 0:
                        ob_cur[0] = pco.tile([P, 4, ob_max], bf16, tag="ob",
                                             name="ob")
                        ob_cur[1] = base
                    ob = ob_cur[0]
                    for (a, b) in rngs:
                        pool = (pools or [psC])[po_ctr[0] % len(pools or [psC])]
                        po_ctr[0] += 1
                        po = pool.tile([P, 512], f32, tag="po", name="po")
                        for f in range(GQ):
                            nc.tensor.matmul(
                                po[:, :b - a],
                                lhsT=wt[:, f, :],
                                rhs=oT_all[:, f, a:b],
                                start=(f == 0), stop=(f == GQ - 1))
                        dst = ob[:, ht % 4, a - base:b - base]
                        if ht % 2 == 0:
                            nc.vector.tensor_copy(out=dst, in_=po[:, :b - a])
                        else:
                            nc.scalar.copy(out=dst, in_=po[:, :b - a])
                    if ht % 4 == 3:
                        ctot = rngs[-1][1] - base
                        nc.sync.dma_start(
                            out=outT[(ht - 3) * P:(ht + 1) * P,
                                     base:base + ctot]
                            .rearrange("(o p) n -> p o n", p=P),
                            in_=ob[:, :, :ctot])

                # ---- attention for one (c, h) --------------------------
                psS_ctx = tc.tile_pool(name="psS", bufs=2, space="PSUM")
                psS = psS_ctx.__enter__()
                psO_ctx = tc.tile_pool(name="psO", bufs=1, space="PSUM")
                psO = psO_ctx.__enter__()

                def emit_b(c, h, inject):
                    nsl = slice(c * N2, (c + 1) * N2)
                    o_ps = psO.tile([P, N2], f32, tag="o", name="o_ps")
                    acc = pb.tile([P, N2], bf16, tag="acc", bufs=2,
                                  name="acc")
                    for m in range(NB):
                        s_ps = psS.tile([P, N2], f32, tag="s", name="s_ps")
                        for u in range(N2 // 512):
                            nc.tensor.matmul(
                                s_ps[:, u * 512:(u + 1) * 512],
                                lhsT=qkT[:, GQ, m * P:(m + 1) * P],
                                rhs=qkT[:, h, c * N2 + u * 512:
                                        c * N2 + (u + 1) * 512],
                                start=True, stop=True)
                        pT = pb.tile([P, N2], bf16, tag="pT", bufs=3,
                                     name="pT")
                        nc.scalar.activation(pT[:], s_ps[:], AF.Exp,
                                             scale=SCALE)
                        for u in range(N2 // 512):
                            usl = slice(u * 512, (u + 1) * 512)
                            nc.tensor.matmul(
                                o_ps[:, usl], lhsT=v_all[:, m, :],
                                rhs=pT[:, usl],
                                start=(m == 0), stop=(m == NB - 1))
                        if m == 0:
                            nc.vector.tensor_copy(out=acc[:], in_=pT[:])
                        else:
                            nc.vector.tensor_tensor(acc[:], acc[:],
                                                    pT[:], OP.add)
                        if inject is not None:
                            inject()
                    # denominator: ones-matmul column-sums acc; the 128-wide
                    # ones stationary broadcasts the row to every partition.
                    # sig/den = exp(-(ln den - ln sig)) avoids the slow DVE
                    # reciprocal; exp+ln live in one scalar table set.
                    den_ps = psS.tile([P, N2], f32, tag="s", name="den_ps")
                    for u in range(N2 // 512):
                        usl = slice(u * 512, (u + 1) * 512)
                        nc.tensor.matmul(den_ps[:, usl], lhsT=ones_bf[:],
                                         rhs=acc[:, usl],
                                         start=True, stop=True)
                    lt = pb.tile([P, N2], f32, tag="lt", bufs=2, name="lt")
                    nc.scalar.activation(lt[:], den_ps[:], AF.Ln)
                    nc.vector.tensor_tensor(lt[:], lt[:],
                                            lsig_b[h][:, nsl], OP.subtract)
                    dg = pb.tile([P, N2], bf16, tag="dg", bufs=2, name="dg")
                    nc.scalar.activation(dg[:], lt[:], AF.Exp, scale=-1.0)
                    nc.vector.tensor_tensor(oT_all[:, h, nsl], o_ps[:],
                                            dg[:], OP.mult)

                # ---- fused schedule ------------------------------------
                # deferred q transposes (tokens >= N2) fill B(c0) PE gaps
                dq_work = [(tok0, h) for tok0 in sorted(defer_q)
                           for h in range(GQ)]
                dctr = [0]

                def inject_defer():
                    if dctr[0] < len(dq_work):
                        tok0, h = dq_work[dctr[0]]
                        dctr[0] += 1
                        qrb, nt = defer_q[tok0]
                        tp = psC.tile([P, P], bf16, tag="po", name="tp")
                        nc.tensor.transpose(tp[:, :nt], qrb[:nt, h, :],
                                            ident_bf[:nt, :nt])
                        nc.vector.tensor_copy(
                            out=qkT[:, h, tok0:tok0 + nt], in_=tp[:, :nt])

                for h in range(GQ):
                    emit_b(0, h, inject_defer)
                while dctr[0] < len(dq_work):
                    inject_defer()
                for k in range(4):
                    c_prefetch(k, cunits_s0)
                ctr = [0]

                def inject_s0():
                    if ctr[0] < len(cunits_s0):
                        emit_c_unit(ctr[0], cunits_s0)
                        ctr[0] += 1

                for h in range(GQ):
                    emit_b(1, h, inject_s0)
                while ctr[0] < len(cunits_s0):
                    inject_s0()
                # B pools released -> extra PSUM double-buffering for the
                # projection tail; DMA issues split over both HWDGE queues
                psO_ctx.__exit__(None, None, None)
                psS_ctx.__exit__(None, None, None)
                psC2_ctx = tc.tile_pool(name="psC2", bufs=2, space="PSUM")
                psC2 = psC2_ctx.__enter__()

                def engf_alt(k):
                    return nc.scalar if k % 2 else nc.sync

                for k in range(4):
                    c_prefetch(k, cunits_s1, engf_alt)
                for k in range(len(cunits_s1)):
                    emit_c_unit(k, cunits_s1, engf_alt, [psC, psC2])
                psC2_ctx.__exit__(None, None, None)

    return nc, tiles, xt_offs, xt_total


# ---------------------------------------------------------------------------
# host wrapper
# ---------------------------------------------------------------------------

def prepare(hidden_states, rope, pre_norm_w, qkv_w, q_norm_w, k_norm_w,
            proj_w, modality_ids):
    """Host-side layout prep. Returns (counts, perm, in_maps_fn) where
    in_maps_fn(tiles, xt_offs, xt_total) builds the per-core input maps."""
    import ml_dtypes

    bf16 = ml_dtypes.bfloat16
    x = np.asarray(hidden_states, np.float32)
    rope = np.asarray(rope, np.float32)
    pre_w = np.asarray(pre_norm_w, np.float32).reshape(NUM_MOD, HIDDEN)
    qkv_w = np.asarray(qkv_w, np.float32).reshape(NUM_MOD, QKV_OUT, HIDDEN)
    qn_w = np.asarray(q_norm_w, np.float32).reshape(NUM_MOD, HEAD_DIM)
    kn_w = np.asarray(k_norm_w, np.float32).reshape(NUM_MOD, HEAD_DIM)
    proj_w = np.asarray(proj_w, np.float32).reshape(NUM_MOD, HIDDEN, Q_SIZE)
    mids = np.asarray(modality_ids).astype(np.int64)

    perm = np.argsort(mids, kind="stable")
    counts = tuple(int((mids == g).sum()) for g in range(NUM_MOD))
    x_p = x[perm]
    rope_p = rope[perm]
    mids_p = mids[perm]

    # ---- rope coefficient tables (fold q/k-norm w+1) ----
    sin = rope_p[:, :64]
    cos = rope_p[:, 64:]
    wq = qn_w[mids_p] + 1.0                             # [N, 128]
    wk = kn_w[mids_p] + 1.0
    ropec = np.empty((N_TOK, 8, 64), np.float32)
    ropec[:, 0] = cos * wq[:, :64]
    ropec[:, 1] = sin * wq[:, 64:]
    ropec[:, 2] = sin * wq[:, :64]
    ropec[:, 3] = cos * wq[:, 64:]
    ropec[:, 4] = cos * wk[:, :64]
    ropec[:, 5] = sin * wk[:, 64:]
    ropec[:, 6] = sin * wk[:, :64]
    ropec[:, 7] = cos * wk[:, 64:]

    # ---- per-core weight slices ----
    wqkv_cores = []
    wproj_cores = []
    for c in range(NCORES):
        rows = np.concatenate([
            np.arange(c * QC, (c + 1) * QC),
            np.arange(Q_SIZE + c * HEAD_DIM, Q_SIZE + (c + 1) * HEAD_DIM),
            np.arange(Q_SIZE + KV_SIZE + c * HEAD_DIM,
                      Q_SIZE + KV_SIZE + (c + 1) * HEAD_DIM),
            np.arange(Q_SIZE + 2 * KV_SIZE + c * GQ,
                      Q_SIZE + 2 * KV_SIZE + (c + 1) * GQ),
        ])
        wc = qkv_w[:, rows, :] * (pre_w[:, None, :] + 1.0)  # [3, 901, 5120]
        wt = wc.transpose(0, 2, 1).reshape(NUM_MOD, KO, P, FC)
        wqkv_cores.append(np.ascontiguousarray(wt).astype(bf16))
        # proj slice: [3, 40, 5, 128(d), 128(hcol)]
        pc = proj_w[:, :, c * QC:(c + 1) * QC]              # [3, 5120, 640]
        pt = pc.reshape(NUM_MOD, HT, P, GQ, HEAD_DIM).transpose(0, 1, 3, 4, 2)
        wproj_cores.append(np.ascontiguousarray(pt).astype(bf16))

    x_bf = x_p.astype(bf16)

    def in_maps_fn(tiles, xt_offs, xt_total):
        xt_flat = np.empty(xt_total, bf16)
        for (tok0, nt, g), off in zip(tiles, xt_offs):
            blk = x_bf[tok0:tok0 + nt]                    # [nt, 5120]
            t = blk.reshape(nt, KO, P).transpose(2, 1, 0)  # [p, ko, nt]
            xt_flat[off:off + P * KO * nt] = \
                np.ascontiguousarray(t).reshape(-1)
        return [{
            "xt": xt_flat,
            "xn": x_bf,
            "ropec": ropec,
            "wqkv": wqkv_cores[c],
            "wproj": wproj_cores[c],
        } for c in range(NCORES)]

    return counts, perm, in_maps_fn


def kernel(hidden_states, rope, pre_norm_w, qkv_w, q_norm_w, k_norm_w,
           proj_w, modality_ids):
    global LAST_EXEC_NS

    counts, perm, in_maps_fn = prepare(
        hidden_states, rope, pre_norm_w, qkv_w, q_norm_w, k_norm_w,
        proj_w, modality_ids)

    if counts not in _BUILD_CACHE:
        _install_profile_hook()
        _install_legalizer()
        _BUILD_CACHE[counts] = _build(counts)
    nc, tiles, xt_offs, xt_total = _BUILD_CACHE[counts]

    in_maps = in_maps_fn(tiles, xt_offs, xt_total)

    from concourse.bass_utils import run_bass_kernel_spmd

    trace = os.environ.get("BASSMOE_TRACE", "") == "1"
    res = run_bass_kernel_spmd(nc, in_maps, core_ids=list(range(NCORES)),
                               trace=trace)
    LAST_EXEC_NS = res.exec_time_ns

    acc = np.zeros((HIDDEN, N_TOK), np.float64)
    for c in range(NCORES):
        acc += np.asarray(res.results[c]["outT"], np.float64)
    out_p = acc.T.astype(np.float32)                    # [N, HIDDEN] permuted
    out = np.empty_like(out_p)
    out[perm] = out_p
    return out
